# revision 26
# baseline (speedup 1.0000x reference)
"""DGL-style 2-layer GAT on 8 TRN2 NeuronCores (Bass/Tile), v2.

Sharding: dst nodes + incident edges partitioned across 8 cores; weights
replicated; src features shared via AllGather of G (feat rows).

v2 vs baseline: the per-tile indirect DMAs (994ns SWDGE overhead each,
~1.6ms of serialized GPSIMD) are replaced by batched dma_gather
(InstDMAGatherAnt) — one instruction per (window, row-group) gathering a
whole window of 128-edge tiles.  G rows are 768B (256 feat bf16 + 4 el
f32 + pad) to satisfy dma_gather's 256B-multiple row constraint; int16
gather indices force a split of the node table into <=32768-row groups.
The per-edge er gather is gone entirely: er values live in SBUF per dst
window (computed in phase 1 / gathered once for L2) and are aligned to
edge lanes with a per-tile matmul against a host-precomputed transposed
indicator Mt.  Edge-softmax masking is folded into the aggregation
indicator M (pad edges get rd=255 -> zero row).
"""
import sys
sys.path.insert(0, '/opt/trn_rl_repo')

import numpy as np
import ml_dtypes

import concourse.bass as bass
import concourse.tile as tile
from concourse import bacc, mybir, library_config
from concourse.masks import make_identity

P = 128
NCORES = 8
N0, N1, N2 = 100000, 50000, 8000
E0, E1 = 600000, 80000
F_IN, HID, H, C = 256, 64, 4, 47
NEG = 0.2

BLK1 = N1 // NCORES            # 6250  A/B block size
LPC1 = 2 * BLK1                # 12500 nodes owned per core
LP1 = 12544                    # padded to 98*128
W1N = 49                       # L1 windows per core (6272 dst slots)
DPC1 = W1N * P                 # 6272
BLK2 = N2 // NCORES            # 1000 dst2 per core
W2N = 8                        # L2 windows per core (1024 slots)
DPC2 = W2N * P                 # 1024
GROW1 = 384                    # bf16 slots: 256 feat | 8 el-bitcast | 120 pad (768B)
GROW2 = 256                    # 188 feat | 8 el2 | 8 er2 | 52 pad (512B)
GRP1 = 25088                   # G row groups (4 core-pair blocks)
GRP2 = 32768                   # G2 row groups: [0,32768) and [32768,50176)
Q1START = [0, 3200, 6400, 9472]
Q1SIZE = [3200, 3200, 3072, 3072]

F32 = mybir.dt.float32
BF16 = mybir.dt.bfloat16
I16 = mybir.dt.int16
AF = mybir.ActivationFunctionType
OP = mybir.AluOpType
BF = ml_dtypes.bfloat16

_cache = {}
_last_in_maps = None


def _g1_row(n):
    """Global node id (layer1 src space, 0..N0) -> G row."""
    m = n % N1
    r = m // BLK1
    return LP1 * r + (m - BLK1 * r) + np.where(n < N1, 0, BLK1)


def _g2_row(n):
    """node id (layer2 src space, 0..N1) -> G2 row."""
    r = n // BLK1
    return DPC1 * r + (n - BLK1 * r)


def _pack_layer(g, loc, dst_local, n_win, ngrp):
    """Pack one core's edges of one layer into gather calls.

    g/loc: per-edge gather group id and group-local row.
    Returns (calls, idx_cols, mrd, mt):
      calls: per window list of (group, ntiles)
      idx_cols: [128, 8*Ttot] int16 wrapped gather indices
      mrd:  [P, Ttot] lane->dst-lane (255 for pads)
      mt:   [P, Ttot*128] transposed indicator (d x e), pads zero
    """
    w = dst_local // P
    rd = (dst_local % P).astype(np.int64)
    loc = loc.astype(np.int64)
    order = np.lexsort((g, w))
    w, rd, g, loc = w[order], rd[order], g[order], loc[order]
    calls = []
    idx_chunks = []
    rd_chunks = []
    for wi in range(n_win):
        wcalls = []
        sel_w = w == wi
        if not sel_w.any():
            wcalls.append((0, 1))
            idx_chunks.append(np.zeros(P, np.int64))
            rd_chunks.append(np.full(P, 255, np.int64))
            calls.append(wcalls)
            continue
        for gi in range(ngrp):
            sel = sel_w & (g == gi)
            n = int(sel.sum())
            if n == 0:
                continue
            nt = (n + P - 1) // P
            cap = nt * P
            bi = np.zeros(cap, np.int64)
            bi[:n] = loc[sel]
            br = np.full(cap, 255, np.int64)
            br[:n] = rd[sel]
            wcalls.append((gi, nt))
            idx_chunks.append(bi)
            rd_chunks.append(br)
        calls.append(wcalls)
    idx_flat = np.concatenate(idx_chunks)
    rd_flat = np.concatenate(rd_chunks)
    ttot = len(rd_flat) // P
    # idx wrap: per call, flat i -> [i%16, coloff + i//16]; calls are
    # contiguous col ranges, so the global wrap is per-P*nt chunk -- but the
    # wrap granularity is 16, and each call's cols = nt*8.  Since every call
    # length is a multiple of 128 (>=16), wrapping the whole flat array in
    # one pass per call boundary is identical to wrapping chunks.
    idx_cols = np.zeros((16, ttot * 8), np.int16)
    col0 = 0
    pos = 0
    for wcalls in calls:
        for gi, nt in wcalls:
            nidx = nt * P
            chunk = idx_flat[pos:pos + nidx]
            idx_cols[:, col0:col0 + nidx // 16] = chunk.reshape(nidx // 16, 16).T
            pos += nidx
            col0 += nidx // 16
    idx_cols = np.tile(idx_cols, (8, 1))
    mrd = rd_flat.reshape(ttot, P).T.astype(BF)
    # mt[d, t*128+e] = 1 if rd[t,e]==d
    mt = (rd_flat.reshape(ttot, P)[None, :, :]
          == np.arange(P, dtype=np.int64)[:, None, None]).astype(BF)
    mt = mt.reshape(P, ttot * P)
    return calls, idx_cols, mrd, mt


def build_program(l1_calls, l2_calls, add_b1, add_b2):
    key = (tuple(tuple(wc) for wc in l1_calls),
           tuple(tuple(wc) for wc in l2_calls), add_b1, add_b2)
    if key in _cache:
        return _cache[key]
    t1w = [sum(nt for _, nt in wc) for wc in l1_calls]
    t2w = [sum(nt for _, nt in wc) for wc in l2_calls]
    T1 = sum(t1w)
    T2 = sum(t2w)
    maxT = max(max(t1w), max(t2w))
    nc = bacc.Bacc("TRN2", num_devices=NCORES, num_swdge_queues=4)
    # ---- I/O
    xT = nc.declare_dram_parameter("xT", [F_IN, LP1], BF16, isOutput=False)
    W1e = nc.declare_dram_parameter("W1e", [F_IN, 264], BF16, isOutput=False)
    W2e = nc.declare_dram_parameter("W2e", [F_IN, 196], BF16, isOutput=False)
    b1r = nc.declare_dram_parameter("b1r", [P, 256], F32, isOutput=False)
    b2r = nc.declare_dram_parameter("b2r", [P, C], F32, isOutput=False)
    c2r = nc.declare_dram_parameter("c2r", [P, 196], F32, isOutput=False)
    IDX1 = nc.declare_dram_parameter("IDX1", [P, 8 * T1], I16, isOutput=False)
    MRD1 = nc.declare_dram_parameter("MRD1", [P, T1], BF16, isOutput=False)
    MT1 = nc.declare_dram_parameter("MT1", [P, T1 * P], BF16, isOutput=False)
    IDX2 = nc.declare_dram_parameter("IDX2", [P, 8 * T2], I16, isOutput=False)
    MRD2 = nc.declare_dram_parameter("MRD2", [P, T2], BF16, isOutput=False)
    MT2 = nc.declare_dram_parameter("MT2", [P, T2 * P], BF16, isOutput=False)
    ED2 = nc.declare_dram_parameter("ED2", [P, 8 * W2N], I16, isOutput=False)
    EDH = nc.declare_dram_parameter("EDH", [P, DPC1 // 16], I16, isOutput=False)
    OUT = nc.declare_dram_parameter("OUT", [DPC2, C], F32, isOutput=True)
    # ---- internal DRAM
    Gin = nc.dram_tensor("Gin", [LP1, GROW1], BF16)
    G = nc.dram_tensor("G", [NCORES * LP1, GROW1], BF16, addr_space="Shared")
    G2in = nc.dram_tensor("G2in", [DPC1, GROW2], BF16)
    G2 = nc.dram_tensor("G2", [NCORES * DPC1, GROW2], BF16, addr_space="Shared")

    with tile.TileContext(nc) as tc:
        with (
            tc.tile_pool(name="const", bufs=1) as const,
            tc.tile_pool(name="ps", bufs=2, space="PSUM") as ps,
            tc.tile_pool(name="ps2", bufs=2, space="PSUM") as ps2,
            tc.tile_pool(name="sb", bufs=3) as sb,
        ):
            nc.gpsimd.load_library(library_config.mlp)
            iota_i = const.tile([P, maxT, P], mybir.dt.int32)
            nc.gpsimd.iota(iota_i[:], pattern=[[0, maxT], [1, P]], base=0,
                           channel_multiplier=0)
            iotaT = const.tile([P, maxT, P], BF16)
            nc.vector.tensor_copy(out=iotaT[:], in_=iota_i[:])
            ident = const.tile([P, P], BF16)
            make_identity(nc, ident[:])
            w1t = [const.tile([P, 264], BF16, name=f'w1t{k}') for k in range(2)]
            w2t = [const.tile([P, 196], BF16, name=f'w2t{k}') for k in range(2)]
            for k in range(2):
                nc.sync.dma_start(out=w1t[k][:], in_=W1e[k * P:(k + 1) * P, :])
                nc.sync.dma_start(out=w2t[k][:], in_=W2e[k * P:(k + 1) * P, :])
            b1t = const.tile([P, 256], F32)
            nc.sync.dma_start(out=b1t[:], in_=b1r[:])
            b2t = const.tile([P, C], F32)
            nc.sync.dma_start(out=b2t[:], in_=b2r[:])
            c2t = const.tile([P, 196], F32)
            nc.sync.dma_start(out=c2t[:], in_=c2r[:])
            idx1t = const.tile([P, 8 * T1], I16)
            nc.sync.dma_start(out=idx1t[:], in_=IDX1[:])
            mrd1t = const.tile([P, T1], BF16)
            nc.sync.dma_start(out=mrd1t[:], in_=MRD1[:])
            idx2t = const.tile([P, 8 * T2], I16)
            nc.sync.dma_start(out=idx2t[:], in_=IDX2[:])
            mrd2t = const.tile([P, T2], BF16)
            nc.sync.dma_start(out=mrd2t[:], in_=MRD2[:])
            ed2t = const.tile([P, 8 * W2N], I16)
            nc.sync.dma_start(out=ed2t[:], in_=ED2[:])
            erS = const.tile([P, W1N, 4], BF16)
            er2S = const.tile([P, W2N, 4], BF16)

            # ================= phase 1: feat1 = x @ W1e =================
            g_writes = [[] for _ in range(4)]
            with tc.tile_pool(name="xp", bufs=1) as xp:
                xtq = [[xp.tile([P, Q1SIZE[q]], BF16, name=f'xt{k}q{q}')
                        for q in range(4)] for k in range(2)]
                for k in range(2):
                    for q in range(4):
                        nc.sync.dma_start(
                            out=xtq[k][q][:],
                            in_=xT[k * P:(k + 1) * P,
                                   Q1START[q]:Q1START[q] + Q1SIZE[q]])
                for c in range(LP1 // P):
                    q = 0
                    while c * P >= Q1START[q] + Q1SIZE[q]:
                        q += 1
                    cq = c - Q1START[q] // P
                    pm = ps.tile([P, 264], F32, tag="pfeat")
                    for k in range(2):
                        nc.tensor.matmul(out=pm[:],
                                         lhsT=xtq[k][q][:, cq * P:(cq + 1) * P],
                                         rhs=w1t[k][:],
                                         start=(k == 0), stop=(k == 1))
                    gs = sb.tile([P, GROW1], BF16, tag="gs")
                    nc.vector.tensor_copy(out=gs[:, 0:256], in_=pm[:, 0:256])
                    nc.vector.tensor_copy(
                        out=gs[:, 256:264].bitcast(F32), in_=pm[:, 256:260])
                    d1 = nc.sync.dma_start(out=Gin[c * P:(c + 1) * P, :],
                                           in_=gs[:, :])
                    g_writes[q].append(d1)
                    if c < W1N:
                        nc.vector.tensor_copy(out=erS[:, c, :], in_=pm[:, 260:264])

            # ================= phase 2: AllGather G =================
            cc1 = nc.gpsimd.collective_compute(
                "AllGather", OP.bypass, replica_groups=[list(range(NCORES))],
                ins=[Gin[:]], outs=[G[:]])
            for q in range(4):
                for d in g_writes[q]:
                    tile.add_dep_helper(cc1.ins, d.ins, sync=True)

            # ============ shared edge-phase body ============
            def hoist_er(calls, tws, MTp, ers, mtp, erEall):
                """er-edge alignment for all windows; independent of the
                AllGather, so the PE does it during the collective wait."""
                wt0 = 0
                for w, wcalls in enumerate(calls):
                    T = tws[w]
                    mts = mtp.tile([P, T * P], BF16, tag="mt")
                    nc.sync.dma_start(out=mts[:],
                                      in_=MTp[:, wt0 * P:(wt0 + T) * P])
                    erPS = ps2.tile([P, T, 4], F32, tag="erps")
                    for j in range(T):
                        nc.tensor.matmul(out=erPS[:, j, :],
                                         lhsT=mts[:, j * P:(j + 1) * P],
                                         rhs=ers[:, w, :],
                                         start=True, stop=True)
                    nc.scalar.activation(out=erEall[:, wt0:wt0 + T, :],
                                          in_=erPS[:], func=AF.Copy)
                    wt0 += T

            def edge_phase(calls, tws, idxt, mrdt, MTp, gtabs, grow,
                           nfeat, acc_cols, ers, pools, flush_fn, erEall=None):
                gp, mtp, eep, wfp, mp = pools
                wt0 = 0   # running tile offset
                qn = 0
                for w, wcalls in enumerate(calls):
                    T = tws[w]
                    if erEall is None:
                        mts = mtp.tile([P, T * P], BF16, tag="mt")
                        nc.sync.dma_start(out=mts[:],
                                          in_=MTp[:, wt0 * P:(wt0 + T) * P])
                    gb = gp.tile([P, T, grow], BF16, tag="gb")
                    t0 = 0
                    for gi, nt in wcalls:
                        gtab, gdep = gtabs[gi]
                        gcall = nc.gpsimd.dma_gather(
                            out_ap=gb[:, t0:t0 + nt, :],
                            in_ap=gtab,
                            idxs_ap=idxt[:, 8 * (wt0 + t0):8 * (wt0 + t0 + nt)],
                            num_idxs=nt * P, num_idxs_reg=nt * P,
                            elem_size=grow, queue_num=qn % 4)
                        qn += 1
                        tile.add_dep_helper(gcall.ins, gdep.ins, sync=True)
                        t0 += nt
                    if erEall is None:
                        # er alignment in-loop: erPS[:, j, :] = Mt_j @ er_win
                        erE = ps2.tile([P, T, 4], F32, tag="erps")
                        for j in range(T):
                            nc.tensor.matmul(out=erE[:, j, :],
                                             lhsT=mts[:, j * P:(j + 1) * P],
                                             rhs=ers[:, w, :],
                                             start=True, stop=True)
                        erE = erE[:]
                    else:
                        erE = erEall[:, wt0:wt0 + T, :]
                    eef = eep.tile([P, T, 4], F32, tag="eef")
                    nc.vector.tensor_tensor(
                        out=eef[:],
                        in0=gb[:, :, nfeat:nfeat + 8].bitcast(F32),
                        in1=erE, op=OP.add)
                    # exp(lrelu(x)) == max(exp(x), exp(0.2x)) exactly
                    ex1 = eep.tile([P, T, 4], F32, tag="ex1")
                    nc.scalar.activation(out=ex1[:], in_=eef[:], func=AF.Exp)
                    ex2 = eep.tile([P, T, 4], F32, tag="ex2")
                    nc.scalar.activation(out=ex2[:], in_=eef[:], func=AF.Exp,
                                         scale=NEG)
                    ees = wfp.tile([P, T, nfeat + 4], BF16, tag="ees")
                    nc.vector.tensor_tensor(out=ees[:, :, nfeat:nfeat + 4],
                                            in0=ex1[:], in1=ex2[:], op=OP.max)
                    hd = nfeat // H
                    for h in range(H):
                        nc.vector.tensor_tensor(
                            out=ees[:, :, h * hd:(h + 1) * hd],
                            in0=gb[:, :, h * hd:(h + 1) * hd],
                            in1=ees[:, :, nfeat + h:nfeat + h + 1].broadcast_to(
                                [P, T, hd]),
                            op=OP.mult)
                    mall = mp.tile([P, T, P], BF16, tag="mall")
                    nc.vector.tensor_tensor(
                        out=mall[:], in0=iotaT[:, 0:T, :],
                        in1=mrdt[:, wt0:wt0 + T][:, :, None].broadcast_to(
                            [P, T, P]),
                        op=OP.is_equal)
                    acc = ps.tile([P, acc_cols], F32, tag="acc")
                    for j in range(T):
                        nc.tensor.matmul(out=acc[:], lhsT=mall[:, j, :],
                                         rhs=ees[:, j, :],
                                         start=(j == 0), stop=(j == T - 1))
                    flush_fn(w, acc)
                    wt0 += T

            # ================= phase 3: L1 edge phase =================
            hT = [const.tile([P, DPC1], BF16, name=f'hT{k}') for k in range(2)]

            def flush1(w, acc):
                sden = sb.tile([P, 4], F32, tag="sden")
                nc.vector.tensor_scalar_max(out=sden[:], in0=acc[:, 256:260],
                                            scalar1=1e-30)
                nc.vector.reciprocal(out=sden[:], in_=sden[:])
                z = sb.tile([P, 256], BF16, tag="z")
                nc.vector.tensor_tensor(
                    out=z[:].rearrange("p (h d) -> p h d", h=H),
                    in0=acc[:, 0:256].rearrange("p (h d) -> p h d", h=H),
                    in1=sden[:, :, None].broadcast_to([P, H, HID]), op=OP.mult)
                if add_b1:
                    nc.vector.tensor_tensor(out=z[:], in0=z[:], in1=b1t[:],
                                            op=OP.add)
                # store h+1 = elu(z)+1 = relu(z) + exp(-relu(-z)); the -1
                # is folded into phase 4 as a W2e column-sum correction.
                zm = sb.tile([P, 256], BF16, tag="zm")
                nc.scalar.activation(out=zm[:], in_=z[:], func=AF.Relu,
                                     scale=-1.0)
                nc.scalar.activation(out=zm[:], in_=zm[:], func=AF.Exp,
                                     scale=-1.0)
                hb = sb.tile([P, 256], BF16, tag="hb")
                nc.scalar.activation(out=hb[:], in_=z[:], func=AF.Relu)
                nc.vector.tensor_tensor(out=hb[:], in0=hb[:], in1=zm[:],
                                        op=OP.add)
                for k in range(2):
                    tp = ps.tile([P, P], BF16, tag="tp")
                    nc.tensor.transpose(out=tp[:], in_=hb[:, k * P:(k + 1) * P],
                                        identity=ident[:])
                    nc.vector.tensor_copy(out=hT[k][:, w * P:(w + 1) * P],
                                          in_=tp[:])

            with (
                tc.tile_pool(name="gp", bufs=3) as gp,
                tc.tile_pool(name="mtp", bufs=2) as mtp,
                tc.tile_pool(name="eep", bufs=2) as eep,
                tc.tile_pool(name="wfp", bufs=3) as wfp,
                tc.tile_pool(name="mp", bufs=2) as mp,
            ):
                gsz1 = [GRP1, GRP1, GRP1, NCORES * LP1 - 3 * GRP1]
                erEall = const.tile([P, T1, 4], F32)
                hoist_er(l1_calls, t1w, MT1, erS, mtp, erEall)
                edge_phase(l1_calls, t1w, idx1t, mrd1t, MT1,
                           [(G[q * GRP1:q * GRP1 + gsz1[q]], cc1)
                            for q in range(4)],
                           GROW1, 256, 260, erS,
                           (gp, mtp, eep, wfp, mp), flush1, erEall=erEall)

            # ================= phase 4: feat2 = h @ W2e =================
            g2_writes = []
            for c in range(W1N):
                pm = ps.tile([P, 264], F32, tag="pfeat")
                for k in range(2):
                    nc.tensor.matmul(out=pm[:, 0:196],
                                     lhsT=hT[k][:, c * P:(c + 1) * P],
                                     rhs=w2t[k][:],
                                     start=(k == 0), stop=(k == 1))
                gs = sb.tile([P, GROW2], BF16, tag="gs2")
                nc.vector.tensor_tensor(out=gs[:, 0:188], in0=pm[:, 0:188],
                                        in1=c2t[:, 0:188], op=OP.subtract)
                nc.vector.tensor_tensor(
                    out=gs[:, 188:204].bitcast(F32), in0=pm[:, 188:196],
                    in1=c2t[:, 188:196], op=OP.subtract)
                d1 = nc.sync.dma_start(out=G2in[c * P:(c + 1) * P, :],
                                       in_=gs[:, :])
                g2_writes.append(d1)

            # ================= phase 5: AllGather G2 =================
            cc3 = nc.gpsimd.collective_compute(
                "AllGather", OP.bypass, replica_groups=[list(range(NCORES))],
                ins=[G2in[:]], outs=[G2[:]])
            for d in g2_writes:
                tile.add_dep_helper(cc3.ins, d.ins, sync=True)

            # ===== phase 5b: er2 for my dst2 rows (one gather from G2) =====
            with tc.tile_pool(name="e2p", bufs=1) as e2p:
                g2d = e2p.tile([P, W2N, GROW2], BF16)
                gcall = nc.gpsimd.dma_gather(
                    out_ap=g2d[:], in_ap=G2[0:GRP2], idxs_ap=ed2t[:],
                    num_idxs=W2N * P, num_idxs_reg=W2N * P, elem_size=GROW2)
                tile.add_dep_helper(gcall.ins, cc3.ins, sync=True)
                nc.vector.tensor_copy(out=er2S[:],
                                      in_=g2d[:, :, 196:204].bitcast(F32))

                # ================= phase 6: L2 edge phase =================
                def flush2(w, acc):
                    sden = sb.tile([P, 4], F32, tag="sden2")
                    nc.vector.tensor_scalar_max(out=sden[:],
                                                in0=acc[:, 188:192],
                                                scalar1=1e-30)
                    nc.vector.reciprocal(out=sden[:], in_=sden[:])
                    nc.vector.tensor_scalar_mul(out=sden[:], in0=sden[:],
                                                scalar1=0.25)
                    z = sb.tile([P, 188], F32, tag="z2")
                    nc.vector.tensor_tensor(
                        out=z[:].rearrange("p (h c) -> p h c", h=H),
                        in0=acc[:, 0:188].rearrange("p (h c) -> p h c", h=H),
                        in1=sden[:, :, None].broadcast_to([P, H, C]),
                        op=OP.mult)
                    o = sb.tile([P, C], F32, tag="o")
                    nc.vector.tensor_reduce(
                        out=o[:], in_=z[:].rearrange("p (h c) -> p c h", h=H),
                        axis=mybir.AxisListType.X, op=OP.add)
                    if add_b2:
                        nc.vector.tensor_tensor(out=o[:], in0=o[:], in1=b2t[:],
                                                op=OP.add)
                    nc.sync.dma_start(out=OUT[w * P:(w + 1) * P, :], in_=o[:])

                with (
                    tc.tile_pool(name="gp2", bufs=3) as gp2,
                    tc.tile_pool(name="mtp2", bufs=2) as mtp2,
                    tc.tile_pool(name="eep2", bufs=2) as eep2,
                    tc.tile_pool(name="wfp2", bufs=3) as wfp2,
                    tc.tile_pool(name="mp2", bufs=2) as mp2,
                ):
                    edge_phase(l2_calls, t2w, idx2t, mrd2t, MT2,
                               [(G2[0:GRP2], cc3),
                                (G2[GRP2:NCORES * DPC1], cc3)],
                               GROW2, 188, 192, er2S,
                               (gp2, mtp2, eep2, wfp2, mp2), flush2)

    nc.compile()
    _cache[key] = nc
    return nc


def _run_once(x, W1, al1, ar1, b1, W2, al2, ar2, b2, src0, dst0, src1, dst1):
    def blkdiag(a):  # [H, D] -> [H*D, H]
        out = np.zeros((a.shape[0] * a.shape[1], a.shape[0]), np.float32)
        for h in range(a.shape[0]):
            out[h * a.shape[1]:(h + 1) * a.shape[1], h] = a[h]
        return out

    W1e = np.concatenate([W1, W1 @ blkdiag(al1), W1 @ blkdiag(ar1)],
                         axis=1).astype(BF)
    W2e = np.concatenate([W2, W2 @ blkdiag(al2), W2 @ blkdiag(ar2)],
                         axis=1).astype(BF)
    b1r = np.broadcast_to(b1.reshape(1, 256), (P, 256)).astype(np.float32).copy()
    b2m = b2.reshape(H, C).mean(axis=0)
    b2r = np.broadcast_to(b2m.reshape(1, C), (P, C)).astype(np.float32).copy()
    c2 = W2e.astype(np.float32).sum(axis=0)
    c2r = np.broadcast_to(c2.reshape(1, 196), (P, 196)).astype(np.float32).copy()
    add_b1 = bool(np.any(b1))
    add_b2 = bool(np.any(b2))

    row1 = _g1_row(src0)
    chunk1 = row1 // GRP1
    loc1 = row1 % GRP1
    g2row = _g2_row(src1)
    chunk2 = g2row // GRP2
    loc2 = g2row % GRP2
    core1 = dst0 // BLK1
    core2 = dst1 // BLK2

    in_maps = []
    all_l1_calls = []
    all_l2_calls = []
    packs = []
    for r in range(NCORES):
        sel1 = core1 == r
        c1, i1, m1, t1 = _pack_layer(chunk1[sel1], loc1[sel1],
                                     dst0[sel1] - r * BLK1, W1N, 4)
        sel2 = core2 == r
        c2, i2, m2, t2 = _pack_layer(chunk2[sel2], loc2[sel2],
                                     dst1[sel2] - r * BLK2, W2N, 2)
        all_l1_calls.append(c1)
        all_l2_calls.append(c2)
        packs.append((i1, m1, t1, i2, m2, t2))

    # SPMD: every core runs the same program -> merge call structures by
    # taking, per (window, group), the max tile count across cores.
    def merge_calls(percore, n_win, ngrp):
        merged = []
        for w in range(n_win):
            wc = []
            for gi in range(ngrp):
                nt = 0
                for c in percore:
                    for g_, n_ in c[w]:
                        if g_ == gi:
                            nt = max(nt, n_)
                if nt:
                    wc.append((gi, nt))
            if not wc:
                wc.append((0, 1))
            merged.append(wc)
        return merged

    l1_calls = merge_calls(all_l1_calls, W1N, 4)
    l2_calls = merge_calls(all_l2_calls, W2N, 2)

    # repack per core to the merged structure (pad missing tiles)
    def repack(core_calls, merged, idxc, mrd, mt):
        T = sum(nt for wc in merged for _, nt in wc)
        idx_o = np.zeros((P, 8 * T), np.int16)
        mrd_o = np.full((P, T), 255.0, BF)
        mt_o = np.zeros((P, T * P), BF)
        src_t = 0
        src_map = {}  # (w, g) -> (tile offset, ntiles)
        for w, wc in enumerate(core_calls):
            for g_, n_ in wc:
                src_map[(w, g_)] = (src_t, n_)
                src_t += n_
        dst_t = 0
        for w, wc in enumerate(merged):
            for g_, n_ in wc:
                if (w, g_) in src_map:
                    s0, sn = src_map[(w, g_)]
                    idx_o[:, 8 * dst_t:8 * (dst_t + sn)] = \
                        idxc[:, 8 * s0:8 * (s0 + sn)]
                    mrd_o[:, dst_t:dst_t + sn] = mrd[:, s0:s0 + sn]
                    mt_o[:, P * dst_t:P * (dst_t + sn)] = \
                        mt[:, P * s0:P * (s0 + sn)]
                dst_t += n_
        return idx_o, mrd_o, mt_o

    for r in range(NCORES):
        i1, m1, t1, i2, m2, t2 = packs[r]
        I1, M1, T1m = repack(all_l1_calls[r], l1_calls, i1, m1, t1)
        I2, M2, T2m = repack(all_l2_calls[r], l2_calls, i2, m2, t2)
        # er2 row gather indices: dst2 slot (w,p) -> G2 row of node
        d = np.minimum(1000 * r + np.arange(DPC2), N1 - 1)
        rows = _g2_row(d)
        assert rows.max() < GRP2
        ed = np.zeros((16, 8 * W2N), np.int16)
        for i in range(DPC2):
            ed[i % 16, i // 16] = rows[i]
        ed = np.tile(ed, (8, 1))
        edh = np.zeros((16, DPC1 // 16), np.int16)
        for i in range(DPC1):
            edh[i % 16, i // 16] = i
        edh = np.tile(edh, (8, 1))
        rowsA = np.arange(r * BLK1, (r + 1) * BLK1)
        rowsB = np.arange(N1 + r * BLK1, N1 + (r + 1) * BLK1)
        xT_ = np.zeros((F_IN, LP1), BF)
        xT_[:, :LPC1] = np.concatenate(
            [x[rowsA], x[rowsB]]).T.astype(BF)
        in_maps.append(dict(
            xT=xT_, W1e=W1e, W2e=W2e, b1r=b1r, b2r=b2r, c2r=c2r,
            IDX1=I1, MRD1=M1, MT1=T1m, IDX2=I2, MRD2=M2, MT2=T2m, ED2=ed,
            EDH=edh))

    global _last_in_maps
    _last_in_maps = in_maps
    nc = build_program(l1_calls, l2_calls, add_b1, add_b2)
    from concourse.bass_utils import run_bass_kernel_spmd
    res = None
    last_err = None
    for attempt in range(3):
        try:
            res = run_bass_kernel_spmd(nc, in_maps, core_ids=list(range(NCORES)))
            out = np.concatenate(
                [res.results[r]["OUT"][:BLK2] for r in range(NCORES)], axis=0)
            if np.isnan(out).any() or np.isinf(out).any():
                raise FloatingPointError("nan/inf in kernel output")
            return out.astype(np.float32)
        except Exception as e:
            last_err = e
            import time as _t
            _t.sleep(5)
    raise last_err


def kernel(x, W1, al1, ar1, b1, W2, al2, ar2, b2, src0, dst0, src1, dst1):
    x = np.asarray(x, np.float32); W1 = np.asarray(W1, np.float32)
    al1 = np.asarray(al1, np.float32); ar1 = np.asarray(ar1, np.float32)
    b1 = np.asarray(b1, np.float32); W2 = np.asarray(W2, np.float32)
    al2 = np.asarray(al2, np.float32); ar2 = np.asarray(ar2, np.float32)
    b2 = np.asarray(b2, np.float32)
    src0 = np.asarray(src0, np.int32); dst0 = np.asarray(dst0, np.int32)
    src1 = np.asarray(src1, np.int32); dst1 = np.asarray(dst1, np.int32)
    return _run_once(x, W1, al1, ar1, b1, W2, al2, ar2, b2,
                     src0, dst0, src1, dst1)


# revision 27
# speedup vs baseline: 1.0416x; 1.0416x over previous
"""DGL-style 2-layer GAT on 8 TRN2 NeuronCores (Bass/Tile), v2.

Sharding: dst nodes + incident edges partitioned across 8 cores; weights
replicated; src features shared via AllGather of G (feat rows).

v2 vs baseline: the per-tile indirect DMAs (994ns SWDGE overhead each,
~1.6ms of serialized GPSIMD) are replaced by batched dma_gather
(InstDMAGatherAnt) — one instruction per (window, row-group) gathering a
whole window of 128-edge tiles.  G rows are 768B (256 feat bf16 + 4 el
f32 + pad) to satisfy dma_gather's 256B-multiple row constraint; int16
gather indices force a split of the node table into <=32768-row groups.
The per-edge er gather is gone entirely: er values live in SBUF per dst
window (computed in phase 1 / gathered once for L2) and are aligned to
edge lanes with a per-tile matmul against a host-precomputed transposed
indicator Mt.  Edge-softmax masking is folded into the aggregation
indicator M (pad edges get rd=255 -> zero row).
"""
import sys
sys.path.insert(0, '/opt/trn_rl_repo')

import numpy as np
import ml_dtypes

import concourse.bass as bass
import concourse.tile as tile
from concourse import bacc, mybir, library_config
from concourse.masks import make_identity

P = 128
NCORES = 8
N0, N1, N2 = 100000, 50000, 8000
E0, E1 = 600000, 80000
F_IN, HID, H, C = 256, 64, 4, 47
NEG = 0.2

BLK1 = N1 // NCORES            # 6250  A/B block size
LPC1 = 2 * BLK1                # 12500 nodes owned per core
LP1 = 12544                    # padded to 98*128
W1N = 49                       # L1 windows per core (6272 dst slots)
DPC1 = W1N * P                 # 6272
BLK2 = N2 // NCORES            # 1000 dst2 per core
W2N = 8                        # L2 windows per core (1024 slots)
DPC2 = W2N * P                 # 1024
GROW1 = 384                    # bf16 slots: 256 feat | 8 el-bitcast | 120 pad (768B)
GROW2 = 256                    # 188 feat | 8 el2 | 8 er2 | 52 pad (512B)
GRP1 = 25088                   # G row groups (4 core-pair blocks)
GRP2 = 32768                   # G2 row groups: [0,32768) and [32768,50176)
Q1START = [0, 3200, 6400, 9472]
Q1SIZE = [3200, 3200, 3072, 3072]

F32 = mybir.dt.float32
BF16 = mybir.dt.bfloat16
I16 = mybir.dt.int16
AF = mybir.ActivationFunctionType
OP = mybir.AluOpType
BF = ml_dtypes.bfloat16

_cache = {}
_last_in_maps = None


def _g1_row(n):
    """Global node id (layer1 src space, 0..N0) -> G row."""
    m = n % N1
    r = m // BLK1
    return LP1 * r + (m - BLK1 * r) + np.where(n < N1, 0, BLK1)


def _g2_row(n):
    """node id (layer2 src space, 0..N1) -> G2 row."""
    r = n // BLK1
    return DPC1 * r + (n - BLK1 * r)


def _pack_layer(g, loc, dst_local, n_win, ngrp):
    """Pack one core's edges of one layer into gather calls.

    g/loc: per-edge gather group id and group-local row.
    Returns (calls, idx_cols, mrd, mt):
      calls: per window list of (group, ntiles)
      idx_cols: [128, 8*Ttot] int16 wrapped gather indices
      mrd:  [P, Ttot] lane->dst-lane (255 for pads)
      mt:   [P, Ttot*128] transposed indicator (d x e), pads zero
    """
    w = dst_local // P
    rd = (dst_local % P).astype(np.int64)
    loc = loc.astype(np.int64)
    order = np.lexsort((g, w))
    w, rd, g, loc = w[order], rd[order], g[order], loc[order]
    calls = []
    idx_chunks = []
    rd_chunks = []
    for wi in range(n_win):
        wcalls = []
        sel_w = w == wi
        if not sel_w.any():
            wcalls.append((0, 1))
            idx_chunks.append(np.zeros(P, np.int64))
            rd_chunks.append(np.full(P, 255, np.int64))
            calls.append(wcalls)
            continue
        for gi in range(ngrp):
            sel = sel_w & (g == gi)
            n = int(sel.sum())
            if n == 0:
                continue
            nt = (n + P - 1) // P
            cap = nt * P
            bi = np.zeros(cap, np.int64)
            bi[:n] = loc[sel]
            br = np.full(cap, 255, np.int64)
            br[:n] = rd[sel]
            wcalls.append((gi, nt))
            idx_chunks.append(bi)
            rd_chunks.append(br)
        calls.append(wcalls)
    idx_flat = np.concatenate(idx_chunks)
    rd_flat = np.concatenate(rd_chunks)
    ttot = len(rd_flat) // P
    # idx wrap: per call, flat i -> [i%16, coloff + i//16]; calls are
    # contiguous col ranges, so the global wrap is per-P*nt chunk -- but the
    # wrap granularity is 16, and each call's cols = nt*8.  Since every call
    # length is a multiple of 128 (>=16), wrapping the whole flat array in
    # one pass per call boundary is identical to wrapping chunks.
    idx_cols = np.zeros((16, ttot * 8), np.int16)
    col0 = 0
    pos = 0
    for wcalls in calls:
        for gi, nt in wcalls:
            nidx = nt * P
            chunk = idx_flat[pos:pos + nidx]
            idx_cols[:, col0:col0 + nidx // 16] = chunk.reshape(nidx // 16, 16).T
            pos += nidx
            col0 += nidx // 16
    idx_cols = np.tile(idx_cols, (8, 1))
    mrd = rd_flat.reshape(ttot, P).T.astype(BF)
    # mt[d, t*128+e] = 1 if rd[t,e]==d
    mt = (rd_flat.reshape(ttot, P)[None, :, :]
          == np.arange(P, dtype=np.int64)[:, None, None]).astype(BF)
    mt = mt.reshape(P, ttot * P)
    return calls, idx_cols, mrd, mt


def build_program(l1_calls, l2_calls, add_b1, add_b2):
    key = (tuple(tuple(wc) for wc in l1_calls),
           tuple(tuple(wc) for wc in l2_calls), add_b1, add_b2)
    if key in _cache:
        return _cache[key]
    t1w = [sum(nt for _, nt in wc) for wc in l1_calls]
    t2w = [sum(nt for _, nt in wc) for wc in l2_calls]
    T1 = sum(t1w)
    T2 = sum(t2w)
    maxT = max(max(t1w), max(t2w))
    nc = bacc.Bacc("TRN2", num_devices=NCORES, num_swdge_queues=4)
    # ---- I/O
    xT = nc.declare_dram_parameter("xT", [F_IN, LP1], BF16, isOutput=False)
    W1e = nc.declare_dram_parameter("W1e", [F_IN, 264], BF16, isOutput=False)
    W2e = nc.declare_dram_parameter("W2e", [F_IN, 196], BF16, isOutput=False)
    b1r = nc.declare_dram_parameter("b1r", [P, 256], F32, isOutput=False)
    b2r = nc.declare_dram_parameter("b2r", [P, C], F32, isOutput=False)
    c2r = nc.declare_dram_parameter("c2r", [P, 196], F32, isOutput=False)
    IDX1 = nc.declare_dram_parameter("IDX1", [P, 8 * T1], I16, isOutput=False)
    MRD1 = nc.declare_dram_parameter("MRD1", [P, T1], BF16, isOutput=False)
    MT1 = nc.declare_dram_parameter("MT1", [P, T1 * P], BF16, isOutput=False)
    IDX2 = nc.declare_dram_parameter("IDX2", [P, 8 * T2], I16, isOutput=False)
    MRD2 = nc.declare_dram_parameter("MRD2", [P, T2], BF16, isOutput=False)
    MT2 = nc.declare_dram_parameter("MT2", [P, T2 * P], BF16, isOutput=False)
    ED2 = nc.declare_dram_parameter("ED2", [P, 8 * W2N], I16, isOutput=False)
    EDH = nc.declare_dram_parameter("EDH", [P, DPC1 // 16], I16, isOutput=False)
    OUT = nc.declare_dram_parameter("OUT", [DPC2, C], F32, isOutput=True)
    # ---- internal DRAM
    Gin = nc.dram_tensor("Gin", [LP1, GROW1], BF16)
    G = nc.dram_tensor("G", [NCORES * LP1, GROW1], BF16, addr_space="Shared")
    G2in = nc.dram_tensor("G2in", [DPC1, GROW2], BF16)
    G2 = nc.dram_tensor("G2", [NCORES * DPC1, GROW2], BF16, addr_space="Shared")

    with tile.TileContext(nc) as tc:
        with (
            tc.tile_pool(name="const", bufs=1) as const,
            tc.tile_pool(name="ps", bufs=2, space="PSUM") as ps,
            tc.tile_pool(name="ps2", bufs=2, space="PSUM") as ps2,
            tc.tile_pool(name="sb", bufs=3) as sb,
        ):
            nc.gpsimd.load_library(library_config.mlp)
            iota_i = const.tile([P, maxT, P], mybir.dt.int32)
            nc.gpsimd.iota(iota_i[:], pattern=[[0, maxT], [1, P]], base=0,
                           channel_multiplier=0)
            iotaT = const.tile([P, maxT, P], BF16)
            nc.vector.tensor_copy(out=iotaT[:], in_=iota_i[:])
            ident = const.tile([P, P], BF16)
            make_identity(nc, ident[:])
            w1t = [const.tile([P, 264], BF16, name=f'w1t{k}') for k in range(2)]
            w2t = [const.tile([P, 196], BF16, name=f'w2t{k}') for k in range(2)]
            for k in range(2):
                nc.sync.dma_start(out=w1t[k][:], in_=W1e[k * P:(k + 1) * P, :])
                nc.sync.dma_start(out=w2t[k][:], in_=W2e[k * P:(k + 1) * P, :])
            b1t = const.tile([P, 256], F32)
            nc.sync.dma_start(out=b1t[:], in_=b1r[:])
            b2t = const.tile([P, C], F32)
            nc.sync.dma_start(out=b2t[:], in_=b2r[:])
            c2t = const.tile([P, 196], F32)
            nc.sync.dma_start(out=c2t[:], in_=c2r[:])
            idx1t = const.tile([P, 8 * T1], I16)
            nc.sync.dma_start(out=idx1t[:], in_=IDX1[:])
            mrd1t = const.tile([P, T1], BF16)
            nc.sync.dma_start(out=mrd1t[:], in_=MRD1[:])
            idx2t = const.tile([P, 8 * T2], I16)
            nc.sync.dma_start(out=idx2t[:], in_=IDX2[:])
            mrd2t = const.tile([P, T2], BF16)
            nc.sync.dma_start(out=mrd2t[:], in_=MRD2[:])
            ed2t = const.tile([P, 8 * W2N], I16)
            nc.sync.dma_start(out=ed2t[:], in_=ED2[:])
            erS = const.tile([P, W1N, 4], BF16)
            er2S = const.tile([P, W2N, 4], BF16)

            # ================= phase 1: feat1 = x @ W1e =================
            g_writes = [[] for _ in range(4)]
            with tc.tile_pool(name="xp", bufs=1) as xp:
                xtq = [[xp.tile([P, Q1SIZE[q]], BF16, name=f'xt{k}q{q}')
                        for q in range(4)] for k in range(2)]
                for k in range(2):
                    for q in range(4):
                        nc.sync.dma_start(
                            out=xtq[k][q][:],
                            in_=xT[k * P:(k + 1) * P,
                                   Q1START[q]:Q1START[q] + Q1SIZE[q]])
                for c in range(LP1 // P):
                    q = 0
                    while c * P >= Q1START[q] + Q1SIZE[q]:
                        q += 1
                    cq = c - Q1START[q] // P
                    pm = ps.tile([P, 264], F32, tag="pfeat")
                    for k in range(2):
                        nc.tensor.matmul(out=pm[:],
                                         lhsT=xtq[k][q][:, cq * P:(cq + 1) * P],
                                         rhs=w1t[k][:],
                                         start=(k == 0), stop=(k == 1))
                    gs = sb.tile([P, GROW1], BF16, tag="gs")
                    nc.vector.tensor_copy(out=gs[:, 0:256], in_=pm[:, 0:256])
                    nc.vector.tensor_copy(
                        out=gs[:, 256:264].bitcast(F32), in_=pm[:, 256:260])
                    d1 = nc.sync.dma_start(out=Gin[c * P:(c + 1) * P, :],
                                           in_=gs[:, :])
                    g_writes[q].append(d1)
                    if c < W1N:
                        nc.vector.tensor_copy(out=erS[:, c, :], in_=pm[:, 260:264])

            # ================= phase 2: AllGather G =================
            cc1 = nc.gpsimd.collective_compute(
                "AllGather", OP.bypass, replica_groups=[list(range(NCORES))],
                ins=[Gin[:]], outs=[G[:]])
            for q in range(4):
                for d in g_writes[q]:
                    tile.add_dep_helper(cc1.ins, d.ins, sync=True)

            # ============ shared edge-phase body ============
            def hoist_er(calls, tws, MTp, ers, mtp, erEall):
                """er-edge alignment for all windows; independent of the
                AllGather, so the PE does it during the collective wait."""
                wt0 = 0
                for w, wcalls in enumerate(calls):
                    T = tws[w]
                    mts = mtp.tile([P, T * P], BF16, tag="mt")
                    nc.sync.dma_start(out=mts[:],
                                      in_=MTp[:, wt0 * P:(wt0 + T) * P])
                    erPS = ps2.tile([P, T, 4], F32, tag="erps")
                    for j in range(T):
                        nc.tensor.matmul(out=erPS[:, j, :],
                                         lhsT=mts[:, j * P:(j + 1) * P],
                                         rhs=ers[:, w, :],
                                         start=True, stop=True)
                    nc.scalar.activation(out=erEall[:, wt0:wt0 + T, :],
                                          in_=erPS[:], func=AF.Copy)
                    wt0 += T

            def edge_phase(calls, tws, idxt, mrdt, MTp, gtabs, grow,
                           nfeat, acc_cols, ers, pools, flush_fn, erEall=None):
                gp, mtp, eep, wfp, mp = pools
                wt0 = 0   # running tile offset
                qn = 0
                for w, wcalls in enumerate(calls):
                    T = tws[w]
                    if erEall is None:
                        mts = mtp.tile([P, T * P], BF16, tag="mt")
                        nc.sync.dma_start(out=mts[:],
                                          in_=MTp[:, wt0 * P:(wt0 + T) * P])
                    gb = gp.tile([P, T, grow], BF16, tag="gb")
                    t0 = 0
                    for gi, nt in wcalls:
                        gtab, gdep = gtabs[gi]
                        gcall = nc.gpsimd.dma_gather(
                            out_ap=gb[:, t0:t0 + nt, :],
                            in_ap=gtab,
                            idxs_ap=idxt[:, 8 * (wt0 + t0):8 * (wt0 + t0 + nt)],
                            num_idxs=nt * P, num_idxs_reg=nt * P,
                            elem_size=grow, queue_num=qn % 4)
                        qn += 1
                        tile.add_dep_helper(gcall.ins, gdep.ins, sync=True)
                        t0 += nt
                    if erEall is None:
                        # er alignment in-loop: erPS[:, j, :] = Mt_j @ er_win
                        erE = ps2.tile([P, T, 4], F32, tag="erps")
                        for j in range(T):
                            nc.tensor.matmul(out=erE[:, j, :],
                                             lhsT=mts[:, j * P:(j + 1) * P],
                                             rhs=ers[:, w, :],
                                             start=True, stop=True)
                        erE = erE[:]
                    else:
                        erE = erEall[:, wt0:wt0 + T, :]
                    eef = eep.tile([P, T, 4], F32, tag="eef")
                    nc.vector.tensor_tensor(
                        out=eef[:],
                        in0=gb[:, :, nfeat:nfeat + 8].bitcast(F32),
                        in1=erE, op=OP.add)
                    # exp(lrelu(x)) == max(exp(x), exp(0.2x)) exactly
                    ex1 = eep.tile([P, T, 4], F32, tag="ex1")
                    nc.scalar.activation(out=ex1[:], in_=eef[:], func=AF.Exp)
                    ex2 = eep.tile([P, T, 4], F32, tag="ex2")
                    nc.scalar.activation(out=ex2[:], in_=eef[:], func=AF.Exp,
                                         scale=NEG)
                    ees = wfp.tile([P, T, nfeat + 4], BF16, tag="ees")
                    nc.vector.tensor_tensor(out=ees[:, :, nfeat:nfeat + 4],
                                            in0=ex1[:], in1=ex2[:], op=OP.max)
                    hd = nfeat // H
                    for h in range(H):
                        nc.vector.tensor_tensor(
                            out=ees[:, :, h * hd:(h + 1) * hd],
                            in0=gb[:, :, h * hd:(h + 1) * hd],
                            in1=ees[:, :, nfeat + h:nfeat + h + 1].broadcast_to(
                                [P, T, hd]),
                            op=OP.mult)
                    mall = mp.tile([P, T, P], BF16, tag="mall")
                    nc.vector.tensor_tensor(
                        out=mall[:], in0=iotaT[:, 0:T, :],
                        in1=mrdt[:, wt0:wt0 + T][:, :, None].broadcast_to(
                            [P, T, P]),
                        op=OP.is_equal)
                    acc = ps.tile([P, acc_cols], F32, tag="acc")
                    for j in range(T):
                        nc.tensor.matmul(out=acc[:], lhsT=mall[:, j, :],
                                         rhs=ees[:, j, :],
                                         start=(j == 0), stop=(j == T - 1))
                    flush_fn(w, acc)
                    wt0 += T

            # ================= phase 3: L1 edge phase =================
            hT = [const.tile([P, DPC1], BF16, name=f'hT{k}') for k in range(2)]

            def flush1(w, acc):
                sden = sb.tile([P, 4], F32, tag="sden")
                nc.vector.tensor_scalar_max(out=sden[:], in0=acc[:, 256:260],
                                            scalar1=1e-30)
                nc.vector.reciprocal(out=sden[:], in_=sden[:])
                z = sb.tile([P, 256], BF16, tag="z")
                nc.vector.tensor_tensor(
                    out=z[:].rearrange("p (h d) -> p h d", h=H),
                    in0=acc[:, 0:256].rearrange("p (h d) -> p h d", h=H),
                    in1=sden[:, :, None].broadcast_to([P, H, HID]), op=OP.mult)
                if add_b1:
                    nc.vector.tensor_tensor(out=z[:], in0=z[:], in1=b1t[:],
                                            op=OP.add)
                # store h+1 = elu(z)+1 = relu(z) + exp(-relu(-z)); the -1
                # is folded into phase 4 as a W2e column-sum correction.
                zm = sb.tile([P, 256], BF16, tag="zm")
                nc.scalar.activation(out=zm[:], in_=z[:], func=AF.Relu,
                                     scale=-1.0)
                nc.scalar.activation(out=zm[:], in_=zm[:], func=AF.Exp,
                                     scale=-1.0)
                hb = sb.tile([P, 256], BF16, tag="hb")
                nc.scalar.activation(out=hb[:], in_=z[:], func=AF.Relu)
                nc.vector.tensor_tensor(out=hb[:], in0=hb[:], in1=zm[:],
                                        op=OP.add)
                for k in range(2):
                    tp = ps.tile([P, P], BF16, tag="tp")
                    nc.tensor.transpose(out=tp[:], in_=hb[:, k * P:(k + 1) * P],
                                        identity=ident[:])
                    nc.vector.tensor_copy(out=hT[k][:, w * P:(w + 1) * P],
                                          in_=tp[:])

            with (
                tc.tile_pool(name="gp", bufs=3) as gp,
                tc.tile_pool(name="mtp", bufs=2) as mtp,
                tc.tile_pool(name="eep", bufs=2) as eep,
                tc.tile_pool(name="wfp", bufs=3) as wfp,
                tc.tile_pool(name="mp", bufs=2) as mp,
            ):
                gsz1 = [GRP1, GRP1, GRP1, NCORES * LP1 - 3 * GRP1]
                edge_phase(l1_calls, t1w, idx1t, mrd1t, MT1,
                           [(G[q * GRP1:q * GRP1 + gsz1[q]], cc1)
                            for q in range(4)],
                           GROW1, 256, 260, erS,
                           (gp, mtp, eep, wfp, mp), flush1)

            # ================= phase 4: feat2 = h @ W2e =================
            g2_writes = []
            for c in range(W1N):
                pm = ps.tile([P, 264], F32, tag="pfeat")
                for k in range(2):
                    nc.tensor.matmul(out=pm[:, 0:196],
                                     lhsT=hT[k][:, c * P:(c + 1) * P],
                                     rhs=w2t[k][:],
                                     start=(k == 0), stop=(k == 1))
                gs = sb.tile([P, GROW2], BF16, tag="gs2")
                nc.vector.tensor_tensor(out=gs[:, 0:188], in0=pm[:, 0:188],
                                        in1=c2t[:, 0:188], op=OP.subtract)
                nc.vector.tensor_tensor(
                    out=gs[:, 188:204].bitcast(F32), in0=pm[:, 188:196],
                    in1=c2t[:, 188:196], op=OP.subtract)
                d1 = nc.sync.dma_start(out=G2in[c * P:(c + 1) * P, :],
                                       in_=gs[:, :])
                g2_writes.append(d1)

            # ================= phase 5: AllGather G2 =================
            cc3 = nc.gpsimd.collective_compute(
                "AllGather", OP.bypass, replica_groups=[list(range(NCORES))],
                ins=[G2in[:]], outs=[G2[:]])
            for d in g2_writes:
                tile.add_dep_helper(cc3.ins, d.ins, sync=True)

            # ===== phase 5b: er2 for my dst2 rows (one gather from G2) =====
            with tc.tile_pool(name="e2p", bufs=1) as e2p:
                g2d = e2p.tile([P, W2N, GROW2], BF16)
                gcall = nc.gpsimd.dma_gather(
                    out_ap=g2d[:], in_ap=G2[0:GRP2], idxs_ap=ed2t[:],
                    num_idxs=W2N * P, num_idxs_reg=W2N * P, elem_size=GROW2)
                tile.add_dep_helper(gcall.ins, cc3.ins, sync=True)
                nc.vector.tensor_copy(out=er2S[:],
                                      in_=g2d[:, :, 196:204].bitcast(F32))

                # ================= phase 6: L2 edge phase =================
                def flush2(w, acc):
                    sden = sb.tile([P, 4], F32, tag="sden2")
                    nc.vector.tensor_scalar_max(out=sden[:],
                                                in0=acc[:, 188:192],
                                                scalar1=1e-30)
                    nc.vector.reciprocal(out=sden[:], in_=sden[:])
                    nc.vector.tensor_scalar_mul(out=sden[:], in0=sden[:],
                                                scalar1=0.25)
                    z = sb.tile([P, 188], F32, tag="z2")
                    nc.vector.tensor_tensor(
                        out=z[:].rearrange("p (h c) -> p h c", h=H),
                        in0=acc[:, 0:188].rearrange("p (h c) -> p h c", h=H),
                        in1=sden[:, :, None].broadcast_to([P, H, C]),
                        op=OP.mult)
                    o = sb.tile([P, C], F32, tag="o")
                    nc.vector.tensor_reduce(
                        out=o[:], in_=z[:].rearrange("p (h c) -> p c h", h=H),
                        axis=mybir.AxisListType.X, op=OP.add)
                    if add_b2:
                        nc.vector.tensor_tensor(out=o[:], in0=o[:], in1=b2t[:],
                                                op=OP.add)
                    nc.sync.dma_start(out=OUT[w * P:(w + 1) * P, :], in_=o[:])

                with (
                    tc.tile_pool(name="gp2", bufs=3) as gp2,
                    tc.tile_pool(name="mtp2", bufs=2) as mtp2,
                    tc.tile_pool(name="eep2", bufs=2) as eep2,
                    tc.tile_pool(name="wfp2", bufs=3) as wfp2,
                    tc.tile_pool(name="mp2", bufs=2) as mp2,
                ):
                    edge_phase(l2_calls, t2w, idx2t, mrd2t, MT2,
                               [(G2[0:GRP2], cc3),
                                (G2[GRP2:NCORES * DPC1], cc3)],
                               GROW2, 188, 192, er2S,
                               (gp2, mtp2, eep2, wfp2, mp2), flush2)

    nc.compile()
    _cache[key] = nc
    return nc


def _run_once(x, W1, al1, ar1, b1, W2, al2, ar2, b2, src0, dst0, src1, dst1):
    def blkdiag(a):  # [H, D] -> [H*D, H]
        out = np.zeros((a.shape[0] * a.shape[1], a.shape[0]), np.float32)
        for h in range(a.shape[0]):
            out[h * a.shape[1]:(h + 1) * a.shape[1], h] = a[h]
        return out

    W1e = np.concatenate([W1, W1 @ blkdiag(al1), W1 @ blkdiag(ar1)],
                         axis=1).astype(BF)
    W2e = np.concatenate([W2, W2 @ blkdiag(al2), W2 @ blkdiag(ar2)],
                         axis=1).astype(BF)
    b1r = np.broadcast_to(b1.reshape(1, 256), (P, 256)).astype(np.float32).copy()
    b2m = b2.reshape(H, C).mean(axis=0)
    b2r = np.broadcast_to(b2m.reshape(1, C), (P, C)).astype(np.float32).copy()
    c2 = W2e.astype(np.float32).sum(axis=0)
    c2r = np.broadcast_to(c2.reshape(1, 196), (P, 196)).astype(np.float32).copy()
    add_b1 = bool(np.any(b1))
    add_b2 = bool(np.any(b2))

    row1 = _g1_row(src0)
    chunk1 = row1 // GRP1
    loc1 = row1 % GRP1
    g2row = _g2_row(src1)
    chunk2 = g2row // GRP2
    loc2 = g2row % GRP2
    core1 = dst0 // BLK1
    core2 = dst1 // BLK2

    in_maps = []
    all_l1_calls = []
    all_l2_calls = []
    packs = []
    for r in range(NCORES):
        sel1 = core1 == r
        c1, i1, m1, t1 = _pack_layer(chunk1[sel1], loc1[sel1],
                                     dst0[sel1] - r * BLK1, W1N, 4)
        sel2 = core2 == r
        c2, i2, m2, t2 = _pack_layer(chunk2[sel2], loc2[sel2],
                                     dst1[sel2] - r * BLK2, W2N, 2)
        all_l1_calls.append(c1)
        all_l2_calls.append(c2)
        packs.append((i1, m1, t1, i2, m2, t2))

    # SPMD: every core runs the same program -> merge call structures by
    # taking, per (window, group), the max tile count across cores.
    def merge_calls(percore, n_win, ngrp):
        merged = []
        for w in range(n_win):
            wc = []
            for gi in range(ngrp):
                nt = 0
                for c in percore:
                    for g_, n_ in c[w]:
                        if g_ == gi:
                            nt = max(nt, n_)
                if nt:
                    wc.append((gi, nt))
            if not wc:
                wc.append((0, 1))
            merged.append(wc)
        return merged

    l1_calls = merge_calls(all_l1_calls, W1N, 4)
    l2_calls = merge_calls(all_l2_calls, W2N, 2)

    # repack per core to the merged structure (pad missing tiles)
    def repack(core_calls, merged, idxc, mrd, mt):
        T = sum(nt for wc in merged for _, nt in wc)
        idx_o = np.zeros((P, 8 * T), np.int16)
        mrd_o = np.full((P, T), 255.0, BF)
        mt_o = np.zeros((P, T * P), BF)
        src_t = 0
        src_map = {}  # (w, g) -> (tile offset, ntiles)
        for w, wc in enumerate(core_calls):
            for g_, n_ in wc:
                src_map[(w, g_)] = (src_t, n_)
                src_t += n_
        dst_t = 0
        for w, wc in enumerate(merged):
            for g_, n_ in wc:
                if (w, g_) in src_map:
                    s0, sn = src_map[(w, g_)]
                    idx_o[:, 8 * dst_t:8 * (dst_t + sn)] = \
                        idxc[:, 8 * s0:8 * (s0 + sn)]
                    mrd_o[:, dst_t:dst_t + sn] = mrd[:, s0:s0 + sn]
                    mt_o[:, P * dst_t:P * (dst_t + sn)] = \
                        mt[:, P * s0:P * (s0 + sn)]
                dst_t += n_
        return idx_o, mrd_o, mt_o

    for r in range(NCORES):
        i1, m1, t1, i2, m2, t2 = packs[r]
        I1, M1, T1m = repack(all_l1_calls[r], l1_calls, i1, m1, t1)
        I2, M2, T2m = repack(all_l2_calls[r], l2_calls, i2, m2, t2)
        # er2 row gather indices: dst2 slot (w,p) -> G2 row of node
        d = np.minimum(1000 * r + np.arange(DPC2), N1 - 1)
        rows = _g2_row(d)
        assert rows.max() < GRP2
        ed = np.zeros((16, 8 * W2N), np.int16)
        for i in range(DPC2):
            ed[i % 16, i // 16] = rows[i]
        ed = np.tile(ed, (8, 1))
        edh = np.zeros((16, DPC1 // 16), np.int16)
        for i in range(DPC1):
            edh[i % 16, i // 16] = i
        edh = np.tile(edh, (8, 1))
        rowsA = np.arange(r * BLK1, (r + 1) * BLK1)
        rowsB = np.arange(N1 + r * BLK1, N1 + (r + 1) * BLK1)
        xT_ = np.zeros((F_IN, LP1), BF)
        xT_[:, :LPC1] = np.concatenate(
            [x[rowsA], x[rowsB]]).T.astype(BF)
        in_maps.append(dict(
            xT=xT_, W1e=W1e, W2e=W2e, b1r=b1r, b2r=b2r, c2r=c2r,
            IDX1=I1, MRD1=M1, MT1=T1m, IDX2=I2, MRD2=M2, MT2=T2m, ED2=ed,
            EDH=edh))

    global _last_in_maps
    _last_in_maps = in_maps
    nc = build_program(l1_calls, l2_calls, add_b1, add_b2)
    from concourse.bass_utils import run_bass_kernel_spmd
    res = None
    last_err = None
    for attempt in range(3):
        try:
            res = run_bass_kernel_spmd(nc, in_maps, core_ids=list(range(NCORES)))
            out = np.concatenate(
                [res.results[r]["OUT"][:BLK2] for r in range(NCORES)], axis=0)
            if np.isnan(out).any() or np.isinf(out).any():
                raise FloatingPointError("nan/inf in kernel output")
            return out.astype(np.float32)
        except Exception as e:
            last_err = e
            import time as _t
            _t.sleep(5)
    raise last_err


def kernel(x, W1, al1, ar1, b1, W2, al2, ar2, b2, src0, dst0, src1, dst1):
    x = np.asarray(x, np.float32); W1 = np.asarray(W1, np.float32)
    al1 = np.asarray(al1, np.float32); ar1 = np.asarray(ar1, np.float32)
    b1 = np.asarray(b1, np.float32); W2 = np.asarray(W2, np.float32)
    al2 = np.asarray(al2, np.float32); ar2 = np.asarray(ar2, np.float32)
    b2 = np.asarray(b2, np.float32)
    src0 = np.asarray(src0, np.int32); dst0 = np.asarray(dst0, np.int32)
    src1 = np.asarray(src1, np.int32); dst1 = np.asarray(dst1, np.int32)
    return _run_once(x, W1, al1, ar1, b1, W2, al2, ar2, b2,
                     src0, dst0, src1, dst1)


# revision 28
# speedup vs baseline: 1.1293x; 1.0843x over previous
"""DGL-style 2-layer GAT on 8 TRN2 NeuronCores (Bass/Tile), v2.

Sharding: dst nodes + incident edges partitioned across 8 cores; weights
replicated; src features shared via AllGather of G (feat rows).

v2 vs baseline: the per-tile indirect DMAs (994ns SWDGE overhead each,
~1.6ms of serialized GPSIMD) are replaced by batched dma_gather
(InstDMAGatherAnt) — one instruction per (window, row-group) gathering a
whole window of 128-edge tiles.  G rows are 768B (256 feat bf16 + 4 el
f32 + pad) to satisfy dma_gather's 256B-multiple row constraint; int16
gather indices force a split of the node table into <=32768-row groups.
The per-edge er gather is gone entirely: er values live in SBUF per dst
window (computed in phase 1 / gathered once for L2) and are aligned to
edge lanes with a per-tile matmul against a host-precomputed transposed
indicator Mt.  Edge-softmax masking is folded into the aggregation
indicator M (pad edges get rd=255 -> zero row).
"""
import sys
sys.path.insert(0, '/opt/trn_rl_repo')

import numpy as np
import ml_dtypes

import concourse.bass as bass
import concourse.tile as tile
from concourse import bacc, mybir, library_config
from concourse.masks import make_identity

P = 128
NCORES = 8
N0, N1, N2 = 100000, 50000, 8000
E0, E1 = 600000, 80000
F_IN, HID, H, C = 256, 64, 4, 47
NEG = 0.2

BLK1 = N1 // NCORES            # 6250  A/B block size
LPC1 = 2 * BLK1                # 12500 nodes owned per core
LP1 = 12544                    # padded to 98*128
W1N = 49                       # L1 windows per core (6272 dst slots)
DPC1 = W1N * P                 # 6272
BLK2 = N2 // NCORES            # 1000 dst2 per core
W2N = 8                        # L2 windows per core (1024 slots)
DPC2 = W2N * P                 # 1024
GROW1 = 384                    # bf16 slots: 256 feat | 8 el-bitcast | 120 pad (768B)
GROW2 = 256                    # 188 feat | 8 el2 | 8 er2 | 52 pad (512B)
GRP1 = 25088                   # G row groups (4 core-pair blocks)
GRP2 = 32768                   # G2 row groups: [0,32768) and [32768,50176)
Q1START = [0, 3200, 6400, 9472]
Q1SIZE = [3200, 3200, 3072, 3072]

F32 = mybir.dt.float32
BF16 = mybir.dt.bfloat16
I16 = mybir.dt.int16
AF = mybir.ActivationFunctionType
OP = mybir.AluOpType
BF = ml_dtypes.bfloat16

_cache = {}
_last_in_maps = None


def _g1_row(n):
    """Global node id (layer1 src space, 0..N0) -> G row."""
    m = n % N1
    r = m // BLK1
    return LP1 * r + (m - BLK1 * r) + np.where(n < N1, 0, BLK1)


def _g2_row(n):
    """node id (layer2 src space, 0..N1) -> G2 row."""
    r = n // BLK1
    return DPC1 * r + (n - BLK1 * r)


def _pack_layer(g, loc, dst_local, n_win, ngrp):
    """Pack one core's edges of one layer into gather calls.

    g/loc: per-edge gather group id and group-local row.
    Returns (calls, idx_cols, mrd, mt):
      calls: per window list of (group, ntiles)
      idx_cols: [128, 8*Ttot] int16 wrapped gather indices
      mrd:  [P, Ttot] lane->dst-lane (255 for pads)
      mt:   [P, Ttot*128] transposed indicator (d x e), pads zero
    """
    w = dst_local // P
    rd = (dst_local % P).astype(np.int64)
    loc = loc.astype(np.int64)
    order = np.lexsort((g, w))
    w, rd, g, loc = w[order], rd[order], g[order], loc[order]
    calls = []
    idx_chunks = []
    rd_chunks = []
    for wi in range(n_win):
        wcalls = []
        sel_w = w == wi
        if not sel_w.any():
            wcalls.append((0, 1))
            idx_chunks.append(np.zeros(P, np.int64))
            rd_chunks.append(np.full(P, 255, np.int64))
            calls.append(wcalls)
            continue
        for gi in range(ngrp):
            sel = sel_w & (g == gi)
            n = int(sel.sum())
            if n == 0:
                continue
            nt = (n + P - 1) // P
            cap = nt * P
            bi = np.zeros(cap, np.int64)
            bi[:n] = loc[sel]
            br = np.full(cap, 255, np.int64)
            br[:n] = rd[sel]
            wcalls.append((gi, nt))
            idx_chunks.append(bi)
            rd_chunks.append(br)
        calls.append(wcalls)
    idx_flat = np.concatenate(idx_chunks)
    rd_flat = np.concatenate(rd_chunks)
    ttot = len(rd_flat) // P
    # idx wrap: per call, flat i -> [i%16, coloff + i//16]; calls are
    # contiguous col ranges, so the global wrap is per-P*nt chunk -- but the
    # wrap granularity is 16, and each call's cols = nt*8.  Since every call
    # length is a multiple of 128 (>=16), wrapping the whole flat array in
    # one pass per call boundary is identical to wrapping chunks.
    idx_cols = np.zeros((16, ttot * 8), np.int16)
    col0 = 0
    pos = 0
    for wcalls in calls:
        for gi, nt in wcalls:
            nidx = nt * P
            chunk = idx_flat[pos:pos + nidx]
            idx_cols[:, col0:col0 + nidx // 16] = chunk.reshape(nidx // 16, 16).T
            pos += nidx
            col0 += nidx // 16
    idx_cols = np.tile(idx_cols, (8, 1))
    mrd = rd_flat.reshape(ttot, P).T.astype(BF)
    # mt[d, t*128+e] = 1 if rd[t,e]==d
    mt = (rd_flat.reshape(ttot, P)[None, :, :]
          == np.arange(P, dtype=np.int64)[:, None, None]).astype(BF)
    mt = mt.reshape(P, ttot * P)
    return calls, idx_cols, mrd, mt


def build_program(l1_calls, l2_calls, add_b1, add_b2):
    key = (tuple(tuple(wc) for wc in l1_calls),
           tuple(tuple(wc) for wc in l2_calls), add_b1, add_b2)
    if key in _cache:
        return _cache[key]
    t1w = [sum(nt for _, nt in wc) for wc in l1_calls]
    t2w = [sum(nt for _, nt in wc) for wc in l2_calls]
    T1 = sum(t1w)
    T2 = sum(t2w)
    maxT = max(max(t1w), max(t2w))
    nc = bacc.Bacc("TRN2", num_devices=NCORES, num_swdge_queues=4)
    # ---- I/O
    xT = nc.declare_dram_parameter("xT", [F_IN, LP1], BF16, isOutput=False)
    W1e = nc.declare_dram_parameter("W1e", [F_IN, 264], BF16, isOutput=False)
    W2e = nc.declare_dram_parameter("W2e", [F_IN, 196], BF16, isOutput=False)
    b1r = nc.declare_dram_parameter("b1r", [P, 256], F32, isOutput=False)
    b2r = nc.declare_dram_parameter("b2r", [P, C], F32, isOutput=False)
    c2r = nc.declare_dram_parameter("c2r", [P, 196], F32, isOutput=False)
    IDX1 = nc.declare_dram_parameter("IDX1", [P, 8 * T1], I16, isOutput=False)
    MRD1 = nc.declare_dram_parameter("MRD1", [P, T1], BF16, isOutput=False)
    MT1 = nc.declare_dram_parameter("MT1", [P, T1 * P], BF16, isOutput=False)
    IDX2 = nc.declare_dram_parameter("IDX2", [P, 8 * T2], I16, isOutput=False)
    MRD2 = nc.declare_dram_parameter("MRD2", [P, T2], BF16, isOutput=False)
    MT2 = nc.declare_dram_parameter("MT2", [P, T2 * P], BF16, isOutput=False)
    ED2 = nc.declare_dram_parameter("ED2", [P, 8 * W2N], I16, isOutput=False)
    EDH = nc.declare_dram_parameter("EDH", [P, DPC1 // 16], I16, isOutput=False)
    OUT = nc.declare_dram_parameter("OUT", [DPC2, C], F32, isOutput=True)
    # ---- internal DRAM
    Gin = nc.dram_tensor("Gin", [LP1, GROW1], BF16)
    G = nc.dram_tensor("G", [NCORES * LP1, GROW1], BF16, addr_space="Shared")
    G2in = nc.dram_tensor("G2in", [DPC1, GROW2], BF16)
    G2 = nc.dram_tensor("G2", [NCORES * DPC1, GROW2], BF16, addr_space="Shared")

    with tile.TileContext(nc) as tc:
        with (
            tc.tile_pool(name="const", bufs=1) as const,
            tc.tile_pool(name="ps", bufs=2, space="PSUM") as ps,
            tc.tile_pool(name="ps2", bufs=2, space="PSUM") as ps2,
            tc.tile_pool(name="sb", bufs=3) as sb,
        ):
            nc.gpsimd.load_library(library_config.mlp)
            iota_i = const.tile([P, maxT, P], mybir.dt.int32)
            nc.gpsimd.iota(iota_i[:], pattern=[[0, maxT], [1, P]], base=0,
                           channel_multiplier=0)
            iotaT = const.tile([P, maxT, P], BF16)
            nc.vector.tensor_copy(out=iotaT[:], in_=iota_i[:])
            ident = const.tile([P, P], BF16)
            make_identity(nc, ident[:])
            w1t = [const.tile([P, 264], BF16, name=f'w1t{k}') for k in range(2)]
            w2t = [const.tile([P, 196], BF16, name=f'w2t{k}') for k in range(2)]
            for k in range(2):
                nc.sync.dma_start(out=w1t[k][:], in_=W1e[k * P:(k + 1) * P, :])
                nc.sync.dma_start(out=w2t[k][:], in_=W2e[k * P:(k + 1) * P, :])
            b1t = const.tile([P, 256], F32)
            nc.sync.dma_start(out=b1t[:], in_=b1r[:])
            b2t = const.tile([P, C], F32)
            nc.sync.dma_start(out=b2t[:], in_=b2r[:])
            c2t = const.tile([P, 196], F32)
            nc.sync.dma_start(out=c2t[:], in_=c2r[:])
            idx1t = const.tile([P, 8 * T1], I16)
            nc.sync.dma_start(out=idx1t[:], in_=IDX1[:])
            mrd1t = const.tile([P, T1], BF16)
            nc.sync.dma_start(out=mrd1t[:], in_=MRD1[:])
            idx2t = const.tile([P, 8 * T2], I16)
            nc.sync.dma_start(out=idx2t[:], in_=IDX2[:])
            mrd2t = const.tile([P, T2], BF16)
            nc.sync.dma_start(out=mrd2t[:], in_=MRD2[:])
            ed2t = const.tile([P, 8 * W2N], I16)
            nc.sync.dma_start(out=ed2t[:], in_=ED2[:])
            erS = const.tile([P, W1N, 4], BF16)
            er2S = const.tile([P, W2N, 4], BF16)

            # ================= phase 1: feat1 = x @ W1e =================
            g_writes = [[] for _ in range(4)]
            with tc.tile_pool(name="xp", bufs=1) as xp:
                xtq = [[xp.tile([P, Q1SIZE[q]], BF16, name=f'xt{k}q{q}')
                        for q in range(4)] for k in range(2)]
                for k in range(2):
                    for q in range(4):
                        nc.sync.dma_start(
                            out=xtq[k][q][:],
                            in_=xT[k * P:(k + 1) * P,
                                   Q1START[q]:Q1START[q] + Q1SIZE[q]])
                for c in range(LP1 // P):
                    q = 0
                    while c * P >= Q1START[q] + Q1SIZE[q]:
                        q += 1
                    cq = c - Q1START[q] // P
                    pm = ps.tile([P, 264], F32, tag="pfeat")
                    for k in range(2):
                        nc.tensor.matmul(out=pm[:],
                                         lhsT=xtq[k][q][:, cq * P:(cq + 1) * P],
                                         rhs=w1t[k][:],
                                         start=(k == 0), stop=(k == 1))
                    gs = sb.tile([P, GROW1], BF16, tag="gs")
                    nc.vector.tensor_copy(out=gs[:, 0:256], in_=pm[:, 0:256])
                    nc.vector.tensor_copy(
                        out=gs[:, 256:264].bitcast(F32), in_=pm[:, 256:260])
                    d1 = nc.sync.dma_start(out=Gin[c * P:(c + 1) * P, :],
                                           in_=gs[:, :])
                    g_writes[q].append(d1)
                    if c < W1N:
                        nc.vector.tensor_copy(out=erS[:, c, :], in_=pm[:, 260:264])

            # ================= phase 2: AllGather G =================
            cc1 = nc.gpsimd.collective_compute(
                "AllGather", OP.bypass, replica_groups=[list(range(NCORES))],
                ins=[Gin[:]], outs=[G[:]])
            for q in range(4):
                for d in g_writes[q]:
                    tile.add_dep_helper(cc1.ins, d.ins, sync=True)

            # ============ shared edge-phase body ============
            def hoist_er(calls, tws, MTp, ers, mtp, erEall):
                """er-edge alignment for all windows; independent of the
                AllGather, so the PE does it during the collective wait."""
                wt0 = 0
                for w, wcalls in enumerate(calls):
                    T = tws[w]
                    mts = mtp.tile([P, T * P], BF16, tag="mt")
                    nc.sync.dma_start(out=mts[:],
                                      in_=MTp[:, wt0 * P:(wt0 + T) * P])
                    erPS = ps2.tile([P, T, 4], F32, tag="erps")
                    for j in range(T):
                        nc.tensor.matmul(out=erPS[:, j, :],
                                         lhsT=mts[:, j * P:(j + 1) * P],
                                         rhs=ers[:, w, :],
                                         start=True, stop=True)
                    nc.scalar.activation(out=erEall[:, wt0:wt0 + T, :],
                                          in_=erPS[:], func=AF.Copy)
                    wt0 += T

            def edge_phase(calls, tws, idxt, mrdt, MTp, gtabs, grow,
                           nfeat, acc_cols, ers, pools, flush_fn, erEall=None):
                gp, mtp, eep, wfp, mp = pools
                wt0 = 0   # running tile offset
                qn = 0
                for w, wcalls in enumerate(calls):
                    T = tws[w]
                    if erEall is None:
                        mts = mtp.tile([P, T * P], BF16, tag="mt")
                        nc.sync.dma_start(out=mts[:],
                                          in_=MTp[:, wt0 * P:(wt0 + T) * P])
                    gb = gp.tile([P, T, grow], BF16, tag="gb")
                    t0 = 0
                    for gi, nt in wcalls:
                        gtab, gdep = gtabs[gi]
                        gcall = nc.gpsimd.dma_gather(
                            out_ap=gb[:, t0:t0 + nt, :],
                            in_ap=gtab,
                            idxs_ap=idxt[:, 8 * (wt0 + t0):8 * (wt0 + t0 + nt)],
                            num_idxs=nt * P, num_idxs_reg=nt * P,
                            elem_size=grow, queue_num=qn % 4)
                        qn += 1
                        tile.add_dep_helper(gcall.ins, gdep.ins, sync=True)
                        t0 += nt
                    if erEall is None:
                        # er alignment in-loop: erPS[:, j, :] = Mt_j @ er_win
                        erE = ps2.tile([P, T, 4], F32, tag="erps")
                        for j in range(T):
                            nc.tensor.matmul(out=erE[:, j, :],
                                             lhsT=mts[:, j * P:(j + 1) * P],
                                             rhs=ers[:, w, :],
                                             start=True, stop=True)
                        erE = erE[:]
                    else:
                        erE = erEall[:, wt0:wt0 + T, :]
                    eef = eep.tile([P, T, 4], F32, tag="eef")
                    nc.vector.tensor_tensor(
                        out=eef[:],
                        in0=gb[:, :, nfeat:nfeat + 8].bitcast(F32),
                        in1=erE, op=OP.add)
                    # exp(lrelu(x)) == max(exp(x), exp(0.2x)) exactly
                    ex1 = eep.tile([P, T, 4], F32, tag="ex1")
                    nc.scalar.activation(out=ex1[:], in_=eef[:], func=AF.Exp)
                    ex2 = eep.tile([P, T, 4], F32, tag="ex2")
                    nc.scalar.activation(out=ex2[:], in_=eef[:], func=AF.Exp,
                                         scale=NEG)
                    ees = wfp.tile([P, T, nfeat + 4], BF16, tag="ees")
                    nc.vector.tensor_tensor(out=ees[:, :, nfeat:nfeat + 4],
                                            in0=ex1[:], in1=ex2[:], op=OP.max)
                    hd = nfeat // H
                    for h in range(H):
                        nc.vector.tensor_tensor(
                            out=ees[:, :, h * hd:(h + 1) * hd],
                            in0=gb[:, :, h * hd:(h + 1) * hd],
                            in1=ees[:, :, nfeat + h:nfeat + h + 1].broadcast_to(
                                [P, T, hd]),
                            op=OP.mult)
                    mall = mp.tile([P, T, P], BF16, tag="mall")
                    nc.vector.tensor_tensor(
                        out=mall[:], in0=iotaT[:, 0:T, :],
                        in1=mrdt[:, wt0:wt0 + T][:, :, None].broadcast_to(
                            [P, T, P]),
                        op=OP.is_equal)
                    acc = ps.tile([P, acc_cols], F32, tag="acc")
                    for j in range(T):
                        nc.tensor.matmul(out=acc[:], lhsT=mall[:, j, :],
                                         rhs=ees[:, j, :],
                                         start=(j == 0), stop=(j == T - 1))
                    flush_fn(w, acc)
                    wt0 += T

            # ================= phase 3: L1 edge phase =================
            hT = [const.tile([P, DPC1], BF16, name=f'hT{k}') for k in range(2)]
            g2_writes = []

            def flush1(w, acc):
                sden = sb.tile([P, 4], F32, tag="sden")
                nc.vector.tensor_scalar_max(out=sden[:], in0=acc[:, 256:260],
                                            scalar1=1e-30)
                nc.vector.reciprocal(out=sden[:], in_=sden[:])
                z = sb.tile([P, 256], BF16, tag="z")
                nc.vector.tensor_tensor(
                    out=z[:].rearrange("p (h d) -> p h d", h=H),
                    in0=acc[:, 0:256].rearrange("p (h d) -> p h d", h=H),
                    in1=sden[:, :, None].broadcast_to([P, H, HID]), op=OP.mult)
                if add_b1:
                    nc.vector.tensor_tensor(out=z[:], in0=z[:], in1=b1t[:],
                                            op=OP.add)
                # store h+1 = elu(z)+1 = relu(z) + exp(-relu(-z)); the -1
                # is folded into phase 4 as a W2e column-sum correction.
                zm = sb.tile([P, 256], BF16, tag="zm")
                nc.scalar.activation(out=zm[:], in_=z[:], func=AF.Relu,
                                     scale=-1.0)
                nc.scalar.activation(out=zm[:], in_=zm[:], func=AF.Exp,
                                     scale=-1.0)
                hb = sb.tile([P, 256], BF16, tag="hb")
                nc.scalar.activation(out=hb[:], in_=z[:], func=AF.Relu)
                nc.vector.tensor_tensor(out=hb[:], in0=hb[:], in1=zm[:],
                                        op=OP.add)
                for k in range(2):
                    tp = ps.tile([P, P], BF16, tag="tp")
                    nc.tensor.transpose(out=tp[:], in_=hb[:, k * P:(k + 1) * P],
                                        identity=ident[:])
                    nc.vector.tensor_copy(out=hT[k][:, w * P:(w + 1) * P],
                                          in_=tp[:])
                # fused phase 4: feat2 row block for this window
                pm2 = ps.tile([P, 264], F32, tag="pfeat")
                for k in range(2):
                    nc.tensor.matmul(out=pm2[:, 0:196],
                                     lhsT=hT[k][:, w * P:(w + 1) * P],
                                     rhs=w2t[k][:],
                                     start=(k == 0), stop=(k == 1))
                gs2 = sb.tile([P, GROW2], BF16, tag="gs2")
                nc.vector.tensor_tensor(out=gs2[:, 0:188], in0=pm2[:, 0:188],
                                        in1=c2t[:, 0:188], op=OP.subtract)
                nc.vector.tensor_tensor(
                    out=gs2[:, 188:204].bitcast(F32), in0=pm2[:, 188:196],
                    in1=c2t[:, 188:196], op=OP.subtract)
                d1 = nc.sync.dma_start(out=G2in[w * P:(w + 1) * P, :],
                                       in_=gs2[:, :])
                g2_writes.append(d1)

            with (
                tc.tile_pool(name="gp", bufs=3) as gp,
                tc.tile_pool(name="mtp", bufs=2) as mtp,
                tc.tile_pool(name="eep", bufs=2) as eep,
                tc.tile_pool(name="wfp", bufs=3) as wfp,
                tc.tile_pool(name="mp", bufs=2) as mp,
            ):
                gsz1 = [GRP1, GRP1, GRP1, NCORES * LP1 - 3 * GRP1]
                edge_phase(l1_calls, t1w, idx1t, mrd1t, MT1,
                           [(G[q * GRP1:q * GRP1 + gsz1[q]], cc1)
                            for q in range(4)],
                           GROW1, 256, 260, erS,
                           (gp, mtp, eep, wfp, mp), flush1)

            # ================= phase 5: AllGather G2 =================
            cc3 = nc.gpsimd.collective_compute(
                "AllGather", OP.bypass, replica_groups=[list(range(NCORES))],
                ins=[G2in[:]], outs=[G2[:]])
            for d in g2_writes:
                tile.add_dep_helper(cc3.ins, d.ins, sync=True)

            # ===== phase 5b: er2 for my dst2 rows (one gather from G2) =====
            with tc.tile_pool(name="e2p", bufs=1) as e2p:
                g2d = e2p.tile([P, W2N, GROW2], BF16)
                gcall = nc.gpsimd.dma_gather(
                    out_ap=g2d[:], in_ap=G2[0:GRP2], idxs_ap=ed2t[:],
                    num_idxs=W2N * P, num_idxs_reg=W2N * P, elem_size=GROW2)
                tile.add_dep_helper(gcall.ins, cc3.ins, sync=True)
                nc.vector.tensor_copy(out=er2S[:],
                                      in_=g2d[:, :, 196:204].bitcast(F32))

                # ================= phase 6: L2 edge phase =================
                def flush2(w, acc):
                    sden = sb.tile([P, 4], F32, tag="sden2")
                    nc.vector.tensor_scalar_max(out=sden[:],
                                                in0=acc[:, 188:192],
                                                scalar1=1e-30)
                    nc.vector.reciprocal(out=sden[:], in_=sden[:])
                    nc.vector.tensor_scalar_mul(out=sden[:], in0=sden[:],
                                                scalar1=0.25)
                    z = sb.tile([P, 188], F32, tag="z2")
                    nc.vector.tensor_tensor(
                        out=z[:].rearrange("p (h c) -> p h c", h=H),
                        in0=acc[:, 0:188].rearrange("p (h c) -> p h c", h=H),
                        in1=sden[:, :, None].broadcast_to([P, H, C]),
                        op=OP.mult)
                    o = sb.tile([P, C], F32, tag="o")
                    nc.vector.tensor_reduce(
                        out=o[:], in_=z[:].rearrange("p (h c) -> p c h", h=H),
                        axis=mybir.AxisListType.X, op=OP.add)
                    if add_b2:
                        nc.vector.tensor_tensor(out=o[:], in0=o[:], in1=b2t[:],
                                                op=OP.add)
                    nc.sync.dma_start(out=OUT[w * P:(w + 1) * P, :], in_=o[:])

                with (
                    tc.tile_pool(name="gp2", bufs=3) as gp2,
                    tc.tile_pool(name="mtp2", bufs=2) as mtp2,
                    tc.tile_pool(name="eep2", bufs=2) as eep2,
                    tc.tile_pool(name="wfp2", bufs=3) as wfp2,
                    tc.tile_pool(name="mp2", bufs=2) as mp2,
                ):
                    edge_phase(l2_calls, t2w, idx2t, mrd2t, MT2,
                               [(G2[0:GRP2], cc3),
                                (G2[GRP2:NCORES * DPC1], cc3)],
                               GROW2, 188, 192, er2S,
                               (gp2, mtp2, eep2, wfp2, mp2), flush2)

    nc.compile()
    _cache[key] = nc
    return nc


def _run_once(x, W1, al1, ar1, b1, W2, al2, ar2, b2, src0, dst0, src1, dst1):
    def blkdiag(a):  # [H, D] -> [H*D, H]
        out = np.zeros((a.shape[0] * a.shape[1], a.shape[0]), np.float32)
        for h in range(a.shape[0]):
            out[h * a.shape[1]:(h + 1) * a.shape[1], h] = a[h]
        return out

    W1e = np.concatenate([W1, W1 @ blkdiag(al1), W1 @ blkdiag(ar1)],
                         axis=1).astype(BF)
    W2e = np.concatenate([W2, W2 @ blkdiag(al2), W2 @ blkdiag(ar2)],
                         axis=1).astype(BF)
    b1r = np.broadcast_to(b1.reshape(1, 256), (P, 256)).astype(np.float32).copy()
    b2m = b2.reshape(H, C).mean(axis=0)
    b2r = np.broadcast_to(b2m.reshape(1, C), (P, C)).astype(np.float32).copy()
    c2 = W2e.astype(np.float32).sum(axis=0)
    c2r = np.broadcast_to(c2.reshape(1, 196), (P, 196)).astype(np.float32).copy()
    add_b1 = bool(np.any(b1))
    add_b2 = bool(np.any(b2))

    row1 = _g1_row(src0)
    chunk1 = row1 // GRP1
    loc1 = row1 % GRP1
    g2row = _g2_row(src1)
    chunk2 = g2row // GRP2
    loc2 = g2row % GRP2
    core1 = dst0 // BLK1
    core2 = dst1 // BLK2

    in_maps = []
    all_l1_calls = []
    all_l2_calls = []
    packs = []
    for r in range(NCORES):
        sel1 = core1 == r
        c1, i1, m1, t1 = _pack_layer(chunk1[sel1], loc1[sel1],
                                     dst0[sel1] - r * BLK1, W1N, 4)
        sel2 = core2 == r
        c2, i2, m2, t2 = _pack_layer(chunk2[sel2], loc2[sel2],
                                     dst1[sel2] - r * BLK2, W2N, 2)
        all_l1_calls.append(c1)
        all_l2_calls.append(c2)
        packs.append((i1, m1, t1, i2, m2, t2))

    # SPMD: every core runs the same program -> merge call structures by
    # taking, per (window, group), the max tile count across cores.
    def merge_calls(percore, n_win, ngrp):
        merged = []
        for w in range(n_win):
            wc = []
            for gi in range(ngrp):
                nt = 0
                for c in percore:
                    for g_, n_ in c[w]:
                        if g_ == gi:
                            nt = max(nt, n_)
                if nt:
                    wc.append((gi, nt))
            if not wc:
                wc.append((0, 1))
            merged.append(wc)
        return merged

    l1_calls = merge_calls(all_l1_calls, W1N, 4)
    l2_calls = merge_calls(all_l2_calls, W2N, 2)

    # repack per core to the merged structure (pad missing tiles)
    def repack(core_calls, merged, idxc, mrd, mt):
        T = sum(nt for wc in merged for _, nt in wc)
        idx_o = np.zeros((P, 8 * T), np.int16)
        mrd_o = np.full((P, T), 255.0, BF)
        mt_o = np.zeros((P, T * P), BF)
        src_t = 0
        src_map = {}  # (w, g) -> (tile offset, ntiles)
        for w, wc in enumerate(core_calls):
            for g_, n_ in wc:
                src_map[(w, g_)] = (src_t, n_)
                src_t += n_
        dst_t = 0
        for w, wc in enumerate(merged):
            for g_, n_ in wc:
                if (w, g_) in src_map:
                    s0, sn = src_map[(w, g_)]
                    idx_o[:, 8 * dst_t:8 * (dst_t + sn)] = \
                        idxc[:, 8 * s0:8 * (s0 + sn)]
                    mrd_o[:, dst_t:dst_t + sn] = mrd[:, s0:s0 + sn]
                    mt_o[:, P * dst_t:P * (dst_t + sn)] = \
                        mt[:, P * s0:P * (s0 + sn)]
                dst_t += n_
        return idx_o, mrd_o, mt_o

    for r in range(NCORES):
        i1, m1, t1, i2, m2, t2 = packs[r]
        I1, M1, T1m = repack(all_l1_calls[r], l1_calls, i1, m1, t1)
        I2, M2, T2m = repack(all_l2_calls[r], l2_calls, i2, m2, t2)
        # er2 row gather indices: dst2 slot (w,p) -> G2 row of node
        d = np.minimum(1000 * r + np.arange(DPC2), N1 - 1)
        rows = _g2_row(d)
        assert rows.max() < GRP2
        ed = np.zeros((16, 8 * W2N), np.int16)
        for i in range(DPC2):
            ed[i % 16, i // 16] = rows[i]
        ed = np.tile(ed, (8, 1))
        edh = np.zeros((16, DPC1 // 16), np.int16)
        for i in range(DPC1):
            edh[i % 16, i // 16] = i
        edh = np.tile(edh, (8, 1))
        rowsA = np.arange(r * BLK1, (r + 1) * BLK1)
        rowsB = np.arange(N1 + r * BLK1, N1 + (r + 1) * BLK1)
        xT_ = np.zeros((F_IN, LP1), BF)
        xT_[:, :LPC1] = np.concatenate(
            [x[rowsA], x[rowsB]]).T.astype(BF)
        in_maps.append(dict(
            xT=xT_, W1e=W1e, W2e=W2e, b1r=b1r, b2r=b2r, c2r=c2r,
            IDX1=I1, MRD1=M1, MT1=T1m, IDX2=I2, MRD2=M2, MT2=T2m, ED2=ed,
            EDH=edh))

    global _last_in_maps
    _last_in_maps = in_maps
    nc = build_program(l1_calls, l2_calls, add_b1, add_b2)
    from concourse.bass_utils import run_bass_kernel_spmd
    res = None
    last_err = None
    for attempt in range(3):
        try:
            res = run_bass_kernel_spmd(nc, in_maps, core_ids=list(range(NCORES)))
            out = np.concatenate(
                [res.results[r]["OUT"][:BLK2] for r in range(NCORES)], axis=0)
            if np.isnan(out).any() or np.isinf(out).any():
                raise FloatingPointError("nan/inf in kernel output")
            return out.astype(np.float32)
        except Exception as e:
            last_err = e
            import time as _t
            _t.sleep(5)
    raise last_err


def kernel(x, W1, al1, ar1, b1, W2, al2, ar2, b2, src0, dst0, src1, dst1):
    x = np.asarray(x, np.float32); W1 = np.asarray(W1, np.float32)
    al1 = np.asarray(al1, np.float32); ar1 = np.asarray(ar1, np.float32)
    b1 = np.asarray(b1, np.float32); W2 = np.asarray(W2, np.float32)
    al2 = np.asarray(al2, np.float32); ar2 = np.asarray(ar2, np.float32)
    b2 = np.asarray(b2, np.float32)
    src0 = np.asarray(src0, np.int32); dst0 = np.asarray(dst0, np.int32)
    src1 = np.asarray(src1, np.int32); dst1 = np.asarray(dst1, np.int32)
    return _run_once(x, W1, al1, ar1, b1, W2, al2, ar2, b2,
                     src0, dst0, src1, dst1)


# revision 29
# speedup vs baseline: 1.1488x; 1.0173x over previous
"""DGL-style 2-layer GAT on 8 TRN2 NeuronCores (Bass/Tile), v2.

Sharding: dst nodes + incident edges partitioned across 8 cores; weights
replicated; src features shared via AllGather of G (feat rows).

v2 vs baseline: the per-tile indirect DMAs (994ns SWDGE overhead each,
~1.6ms of serialized GPSIMD) are replaced by batched dma_gather
(InstDMAGatherAnt) — one instruction per (window, row-group) gathering a
whole window of 128-edge tiles.  G rows are 768B (256 feat bf16 + 4 el
f32 + pad) to satisfy dma_gather's 256B-multiple row constraint; int16
gather indices force a split of the node table into <=32768-row groups.
The per-edge er gather is gone entirely: er values live in SBUF per dst
window (computed in phase 1 / gathered once for L2) and are aligned to
edge lanes with a per-tile matmul against a host-precomputed transposed
indicator Mt.  Edge-softmax masking is folded into the aggregation
indicator M (pad edges get rd=255 -> zero row).
"""
import sys
sys.path.insert(0, '/opt/trn_rl_repo')

import numpy as np
import ml_dtypes

import concourse.bass as bass
import concourse.tile as tile
from concourse import bacc, mybir, library_config
from concourse.masks import make_identity

P = 128
NCORES = 8
N0, N1, N2 = 100000, 50000, 8000
E0, E1 = 600000, 80000
F_IN, HID, H, C = 256, 64, 4, 47
NEG = 0.2

BLK1 = N1 // NCORES            # 6250  A/B block size
LPC1 = 2 * BLK1                # 12500 nodes owned per core
LP1 = 12544                    # padded to 98*128
W1N = 49                       # L1 windows per core (6272 dst slots)
DPC1 = W1N * P                 # 6272
BLK2 = N2 // NCORES            # 1000 dst2 per core
W2N = 8                        # L2 windows per core (1024 slots)
DPC2 = W2N * P                 # 1024
GROW1 = 384                    # bf16 slots: 256 feat | 8 el-bitcast | 120 pad (768B)
GROW2 = 256                    # 188 feat | 8 el2 | 8 er2 | 52 pad (512B)
GRP1 = 25088                   # G row groups (4 core-pair blocks)
GRP2 = 32768                   # G2 row groups: [0,32768) and [32768,50176)
Q1START = [0, 3200, 6400, 9472]
Q1SIZE = [3200, 3200, 3072, 3072]

F32 = mybir.dt.float32
BF16 = mybir.dt.bfloat16
I16 = mybir.dt.int16
AF = mybir.ActivationFunctionType
OP = mybir.AluOpType
BF = ml_dtypes.bfloat16

_cache = {}
_last_in_maps = None


def _g1_row(n):
    """Global node id (layer1 src space, 0..N0) -> G row."""
    m = n % N1
    r = m // BLK1
    return LP1 * r + (m - BLK1 * r) + np.where(n < N1, 0, BLK1)


def _g2_row(n):
    """node id (layer2 src space, 0..N1) -> G2 row."""
    r = n // BLK1
    return DPC1 * r + (n - BLK1 * r)


def _pack_layer(g, loc, dst_local, n_win, ngrp):
    """Pack one core's edges of one layer into gather calls.

    g/loc: per-edge gather group id and group-local row.
    Returns (calls, idx_cols, mrd, mt):
      calls: per window list of (group, ntiles)
      idx_cols: [128, 8*Ttot] int16 wrapped gather indices
      mrd:  [P, Ttot] lane->dst-lane (255 for pads)
      mt:   [P, Ttot*128] transposed indicator (d x e), pads zero
    """
    w = dst_local // P
    rd = (dst_local % P).astype(np.int64)
    loc = loc.astype(np.int64)
    order = np.lexsort((g, w))
    w, rd, g, loc = w[order], rd[order], g[order], loc[order]
    calls = []
    idx_chunks = []
    rd_chunks = []
    for wi in range(n_win):
        wcalls = []
        sel_w = w == wi
        if not sel_w.any():
            wcalls.append((0, 1))
            idx_chunks.append(np.zeros(P, np.int64))
            rd_chunks.append(np.full(P, 255, np.int64))
            calls.append(wcalls)
            continue
        for gi in range(ngrp):
            sel = sel_w & (g == gi)
            n = int(sel.sum())
            if n == 0:
                continue
            nt = (n + P - 1) // P
            cap = nt * P
            bi = np.zeros(cap, np.int64)
            bi[:n] = loc[sel]
            br = np.full(cap, 255, np.int64)
            br[:n] = rd[sel]
            wcalls.append((gi, nt))
            idx_chunks.append(bi)
            rd_chunks.append(br)
        calls.append(wcalls)
    idx_flat = np.concatenate(idx_chunks)
    rd_flat = np.concatenate(rd_chunks)
    ttot = len(rd_flat) // P
    # idx wrap: per call, flat i -> [i%16, coloff + i//16]; calls are
    # contiguous col ranges, so the global wrap is per-P*nt chunk -- but the
    # wrap granularity is 16, and each call's cols = nt*8.  Since every call
    # length is a multiple of 128 (>=16), wrapping the whole flat array in
    # one pass per call boundary is identical to wrapping chunks.
    idx_cols = np.zeros((16, ttot * 8), np.int16)
    col0 = 0
    pos = 0
    for wcalls in calls:
        for gi, nt in wcalls:
            nidx = nt * P
            chunk = idx_flat[pos:pos + nidx]
            idx_cols[:, col0:col0 + nidx // 16] = chunk.reshape(nidx // 16, 16).T
            pos += nidx
            col0 += nidx // 16
    idx_cols = np.tile(idx_cols, (8, 1))
    mrd = rd_flat.reshape(ttot, P).T.astype(BF)
    # mt[d, t*128+e] = 1 if rd[t,e]==d
    mt = (rd_flat.reshape(ttot, P)[None, :, :]
          == np.arange(P, dtype=np.int64)[:, None, None]).astype(BF)
    mt = mt.reshape(P, ttot * P)
    return calls, idx_cols, mrd, mt


def build_program(l1_calls, l2_calls, add_b1, add_b2):
    key = (tuple(tuple(wc) for wc in l1_calls),
           tuple(tuple(wc) for wc in l2_calls), add_b1, add_b2)
    if key in _cache:
        return _cache[key]
    t1w = [sum(nt for _, nt in wc) for wc in l1_calls]
    t2w = [sum(nt for _, nt in wc) for wc in l2_calls]
    T1 = sum(t1w)
    T2 = sum(t2w)
    maxT = max(max(t1w), max(t2w))
    nc = bacc.Bacc("TRN2", num_devices=NCORES, num_swdge_queues=4)
    # ---- I/O
    xT = nc.declare_dram_parameter("xT", [F_IN, LP1], BF16, isOutput=False)
    W1e = nc.declare_dram_parameter("W1e", [F_IN, 264], BF16, isOutput=False)
    W2e = nc.declare_dram_parameter("W2e", [F_IN, 196], BF16, isOutput=False)
    b1r = nc.declare_dram_parameter("b1r", [P, 256], F32, isOutput=False)
    b2r = nc.declare_dram_parameter("b2r", [P, C], F32, isOutput=False)
    c2r = nc.declare_dram_parameter("c2r", [P, 196], F32, isOutput=False)
    IDX1 = nc.declare_dram_parameter("IDX1", [P, 8 * T1], I16, isOutput=False)
    MRD1 = nc.declare_dram_parameter("MRD1", [P, T1], BF16, isOutput=False)
    MT1 = nc.declare_dram_parameter("MT1", [P, T1 * P], BF16, isOutput=False)
    IDX2 = nc.declare_dram_parameter("IDX2", [P, 8 * T2], I16, isOutput=False)
    MRD2 = nc.declare_dram_parameter("MRD2", [P, T2], BF16, isOutput=False)
    MT2 = nc.declare_dram_parameter("MT2", [P, T2 * P], BF16, isOutput=False)
    ED2 = nc.declare_dram_parameter("ED2", [P, 8 * W2N], I16, isOutput=False)
    EDH = nc.declare_dram_parameter("EDH", [P, DPC1 // 16], I16, isOutput=False)
    OUT = nc.declare_dram_parameter("OUT", [DPC2, C], F32, isOutput=True)
    # ---- internal DRAM
    Gin = nc.dram_tensor("Gin", [LP1, GROW1], BF16)
    G = nc.dram_tensor("G", [NCORES * LP1, GROW1], BF16, addr_space="Shared")
    G2in = nc.dram_tensor("G2in", [DPC1, GROW2], BF16)
    G2 = nc.dram_tensor("G2", [NCORES * DPC1, GROW2], BF16, addr_space="Shared")

    with tile.TileContext(nc) as tc:
        with (
            tc.tile_pool(name="const", bufs=1) as const,
            tc.tile_pool(name="ps", bufs=2, space="PSUM") as ps,
            tc.tile_pool(name="ps2", bufs=2, space="PSUM") as ps2,
            tc.tile_pool(name="sb", bufs=3) as sb,
        ):
            nc.gpsimd.load_library(library_config.mlp)
            iota_i = const.tile([P, maxT, P], mybir.dt.int32)
            nc.gpsimd.iota(iota_i[:], pattern=[[0, maxT], [1, P]], base=0,
                           channel_multiplier=0)
            iotaT = const.tile([P, maxT, P], BF16)
            nc.vector.tensor_copy(out=iotaT[:], in_=iota_i[:])
            ident = const.tile([P, P], BF16)
            make_identity(nc, ident[:])
            w1t = [const.tile([P, 264], BF16, name=f'w1t{k}') for k in range(2)]
            w2t = [const.tile([P, 196], BF16, name=f'w2t{k}') for k in range(2)]
            for k in range(2):
                nc.sync.dma_start(out=w1t[k][:], in_=W1e[k * P:(k + 1) * P, :])
                nc.sync.dma_start(out=w2t[k][:], in_=W2e[k * P:(k + 1) * P, :])
            b1t = const.tile([P, 256], F32)
            nc.sync.dma_start(out=b1t[:], in_=b1r[:])
            b2t = const.tile([P, C], F32)
            nc.sync.dma_start(out=b2t[:], in_=b2r[:])
            c2t = const.tile([P, 196], F32)
            nc.sync.dma_start(out=c2t[:], in_=c2r[:])
            idx1t = const.tile([P, 8 * T1], I16)
            nc.sync.dma_start(out=idx1t[:], in_=IDX1[:])
            mrd1t = const.tile([P, T1], BF16)
            nc.sync.dma_start(out=mrd1t[:], in_=MRD1[:])
            idx2t = const.tile([P, 8 * T2], I16)
            nc.sync.dma_start(out=idx2t[:], in_=IDX2[:])
            mrd2t = const.tile([P, T2], BF16)
            nc.sync.dma_start(out=mrd2t[:], in_=MRD2[:])
            mt2t = const.tile([P, T2 * P], BF16)
            nc.sync.dma_start(out=mt2t[:], in_=MT2[:])
            ed2t = const.tile([P, 8 * W2N], I16)
            nc.sync.dma_start(out=ed2t[:], in_=ED2[:])
            erS = const.tile([P, W1N, 4], BF16)
            er2S = const.tile([P, W2N, 4], BF16)

            # ================= phase 1: feat1 = x @ W1e =================
            g_writes = [[] for _ in range(4)]
            with tc.tile_pool(name="xp", bufs=1) as xp:
                xtq = [[xp.tile([P, Q1SIZE[q]], BF16, name=f'xt{k}q{q}')
                        for q in range(4)] for k in range(2)]
                for k in range(2):
                    for q in range(4):
                        nc.sync.dma_start(
                            out=xtq[k][q][:],
                            in_=xT[k * P:(k + 1) * P,
                                   Q1START[q]:Q1START[q] + Q1SIZE[q]])
                for c in range(LP1 // P):
                    q = 0
                    while c * P >= Q1START[q] + Q1SIZE[q]:
                        q += 1
                    cq = c - Q1START[q] // P
                    pm = ps.tile([P, 264], F32, tag="pfeat")
                    for k in range(2):
                        nc.tensor.matmul(out=pm[:],
                                         lhsT=xtq[k][q][:, cq * P:(cq + 1) * P],
                                         rhs=w1t[k][:],
                                         start=(k == 0), stop=(k == 1))
                    gs = sb.tile([P, GROW1], BF16, tag="gs")
                    nc.vector.tensor_copy(out=gs[:, 0:256], in_=pm[:, 0:256])
                    nc.vector.tensor_copy(
                        out=gs[:, 256:264].bitcast(F32), in_=pm[:, 256:260])
                    d1 = nc.sync.dma_start(out=Gin[c * P:(c + 1) * P, :],
                                           in_=gs[:, :])
                    g_writes[q].append(d1)
                    if c < W1N:
                        nc.vector.tensor_copy(out=erS[:, c, :], in_=pm[:, 260:264])

            # ================= phase 2: AllGather G =================
            cc1 = nc.gpsimd.collective_compute(
                "AllGather", OP.bypass, replica_groups=[list(range(NCORES))],
                ins=[Gin[:]], outs=[G[:]])
            for q in range(4):
                for d in g_writes[q]:
                    tile.add_dep_helper(cc1.ins, d.ins, sync=True)

            # ============ shared edge-phase body ============
            def hoist_er(calls, tws, MTp, ers, mtp, erEall):
                """er-edge alignment for all windows; independent of the
                AllGather, so the PE does it during the collective wait."""
                wt0 = 0
                for w, wcalls in enumerate(calls):
                    T = tws[w]
                    mts = mtp.tile([P, T * P], BF16, tag="mt")
                    nc.sync.dma_start(out=mts[:],
                                      in_=MTp[:, wt0 * P:(wt0 + T) * P])
                    erPS = ps2.tile([P, T, 4], F32, tag="erps")
                    for j in range(T):
                        nc.tensor.matmul(out=erPS[:, j, :],
                                         lhsT=mts[:, j * P:(j + 1) * P],
                                         rhs=ers[:, w, :],
                                         start=True, stop=True)
                    nc.scalar.activation(out=erEall[:, wt0:wt0 + T, :],
                                          in_=erPS[:], func=AF.Copy)
                    wt0 += T

            def edge_phase(calls, tws, idxt, mrdt, MTp, gtabs, grow,
                           nfeat, acc_cols, ers, pools, flush_fn, erEall=None,
                           mtfull=None):
                gp, mtp, eep, wfp, mp = pools
                wt0 = 0   # running tile offset
                qn = 0
                for w, wcalls in enumerate(calls):
                    T = tws[w]
                    if mtfull is not None:
                        mts = mtfull[:, wt0 * P:(wt0 + T) * P]
                    elif erEall is None:
                        mts = mtp.tile([P, T * P], BF16, tag="mt")
                        nc.sync.dma_start(out=mts[:],
                                          in_=MTp[:, wt0 * P:(wt0 + T) * P])
                    gb = gp.tile([P, T, grow], BF16, tag="gb")
                    t0 = 0
                    for gi, nt in wcalls:
                        gtab, gdep = gtabs[gi]
                        gcall = nc.gpsimd.dma_gather(
                            out_ap=gb[:, t0:t0 + nt, :],
                            in_ap=gtab,
                            idxs_ap=idxt[:, 8 * (wt0 + t0):8 * (wt0 + t0 + nt)],
                            num_idxs=nt * P, num_idxs_reg=nt * P,
                            elem_size=grow, queue_num=qn % 4)
                        qn += 1
                        tile.add_dep_helper(gcall.ins, gdep.ins, sync=True)
                        t0 += nt
                    if erEall is None:
                        # er alignment in-loop: erE[:, j, :] = Mt_j @ er_win
                        erE = ps2.tile([P, T, 4], F32, tag="erps")
                        for j in range(T):
                            nc.tensor.matmul(out=erE[:, j, :],
                                             lhsT=mts[:, j * P:(j + 1) * P],
                                             rhs=ers[:, w, :],
                                             start=True, stop=True)
                        erE = erE[:]
                    else:
                        erE = erEall[:, wt0:wt0 + T, :]
                    eef = eep.tile([P, T, 4], F32, tag="eef")
                    nc.vector.tensor_tensor(
                        out=eef[:],
                        in0=gb[:, :, nfeat:nfeat + 8].bitcast(F32),
                        in1=erE, op=OP.add)
                    # exp(lrelu(x)) == max(exp(x), exp(0.2x)) exactly
                    ex1 = eep.tile([P, T, 4], F32, tag="ex1")
                    nc.scalar.activation(out=ex1[:], in_=eef[:], func=AF.Exp)
                    ex2 = eep.tile([P, T, 4], F32, tag="ex2")
                    nc.scalar.activation(out=ex2[:], in_=eef[:], func=AF.Exp,
                                         scale=NEG)
                    ees = wfp.tile([P, T, nfeat + 4], BF16, tag="ees")
                    nc.vector.tensor_tensor(out=ees[:, :, nfeat:nfeat + 4],
                                            in0=ex1[:], in1=ex2[:], op=OP.max)
                    hd = nfeat // H
                    for h in range(H):
                        nc.vector.tensor_tensor(
                            out=ees[:, :, h * hd:(h + 1) * hd],
                            in0=gb[:, :, h * hd:(h + 1) * hd],
                            in1=ees[:, :, nfeat + h:nfeat + h + 1].broadcast_to(
                                [P, T, hd]),
                            op=OP.mult)
                    mall = mp.tile([P, T, P], BF16, tag="mall")
                    nc.vector.tensor_tensor(
                        out=mall[:], in0=iotaT[:, 0:T, :],
                        in1=mrdt[:, wt0:wt0 + T][:, :, None].broadcast_to(
                            [P, T, P]),
                        op=OP.is_equal)
                    acc = ps.tile([P, acc_cols], F32, tag="acc")
                    for j in range(T):
                        nc.tensor.matmul(out=acc[:], lhsT=mall[:, j, :],
                                         rhs=ees[:, j, :],
                                         start=(j == 0), stop=(j == T - 1))
                    flush_fn(w, acc)
                    wt0 += T

            # ================= phase 3: L1 edge phase =================
            hT = [const.tile([P, DPC1], BF16, name=f'hT{k}') for k in range(2)]
            g2_writes = []

            def flush1(w, acc):
                sden = sb.tile([P, 4], F32, tag="sden")
                nc.vector.tensor_scalar_max(out=sden[:], in0=acc[:, 256:260],
                                            scalar1=1e-30)
                nc.vector.reciprocal(out=sden[:], in_=sden[:])
                z = sb.tile([P, 256], BF16, tag="z")
                nc.vector.tensor_tensor(
                    out=z[:].rearrange("p (h d) -> p h d", h=H),
                    in0=acc[:, 0:256].rearrange("p (h d) -> p h d", h=H),
                    in1=sden[:, :, None].broadcast_to([P, H, HID]), op=OP.mult)
                if add_b1:
                    nc.vector.tensor_tensor(out=z[:], in0=z[:], in1=b1t[:],
                                            op=OP.add)
                # store h+1 = elu(z)+1 = relu(z) + exp(-relu(-z)); the -1
                # is folded into phase 4 as a W2e column-sum correction.
                zm = sb.tile([P, 256], BF16, tag="zm")
                nc.scalar.activation(out=zm[:], in_=z[:], func=AF.Relu,
                                     scale=-1.0)
                nc.scalar.activation(out=zm[:], in_=zm[:], func=AF.Exp,
                                     scale=-1.0)
                hb = sb.tile([P, 256], BF16, tag="hb")
                nc.scalar.activation(out=hb[:], in_=z[:], func=AF.Relu)
                nc.vector.tensor_tensor(out=hb[:], in0=hb[:], in1=zm[:],
                                        op=OP.add)
                for k in range(2):
                    tp = ps.tile([P, P], BF16, tag="tp")
                    nc.tensor.transpose(out=tp[:], in_=hb[:, k * P:(k + 1) * P],
                                        identity=ident[:])
                    nc.vector.tensor_copy(out=hT[k][:, w * P:(w + 1) * P],
                                          in_=tp[:])
                # fused phase 4: feat2 row block for this window
                pm2 = ps.tile([P, 264], F32, tag="pfeat")
                for k in range(2):
                    nc.tensor.matmul(out=pm2[:, 0:196],
                                     lhsT=hT[k][:, w * P:(w + 1) * P],
                                     rhs=w2t[k][:],
                                     start=(k == 0), stop=(k == 1))
                gs2 = sb.tile([P, GROW2], BF16, tag="gs2")
                nc.vector.tensor_tensor(out=gs2[:, 0:188], in0=pm2[:, 0:188],
                                        in1=c2t[:, 0:188], op=OP.subtract)
                nc.vector.tensor_tensor(
                    out=gs2[:, 188:204].bitcast(F32), in0=pm2[:, 188:196],
                    in1=c2t[:, 188:196], op=OP.subtract)
                d1 = nc.sync.dma_start(out=G2in[w * P:(w + 1) * P, :],
                                       in_=gs2[:, :])
                g2_writes.append(d1)

            with (
                tc.tile_pool(name="gp", bufs=4) as gp,
                tc.tile_pool(name="mtp", bufs=2) as mtp,
                tc.tile_pool(name="eep", bufs=2) as eep,
                tc.tile_pool(name="wfp", bufs=3) as wfp,
                tc.tile_pool(name="mp", bufs=2) as mp,
            ):
                gsz1 = [GRP1, GRP1, GRP1, NCORES * LP1 - 3 * GRP1]
                edge_phase(l1_calls, t1w, idx1t, mrd1t, MT1,
                           [(G[q * GRP1:q * GRP1 + gsz1[q]], cc1)
                            for q in range(4)],
                           GROW1, 256, 260, erS,
                           (gp, mtp, eep, wfp, mp), flush1)

            # ================= phase 5: AllGather G2 =================
            cc3 = nc.gpsimd.collective_compute(
                "AllGather", OP.bypass, replica_groups=[list(range(NCORES))],
                ins=[G2in[:]], outs=[G2[:]])
            for d in g2_writes:
                tile.add_dep_helper(cc3.ins, d.ins, sync=True)

            # ===== phase 5b: er2 for my dst2 rows (one gather from G2) =====
            with tc.tile_pool(name="e2p", bufs=1) as e2p:
                g2d = e2p.tile([P, W2N, GROW2], BF16)
                gcall = nc.gpsimd.dma_gather(
                    out_ap=g2d[:], in_ap=G2[0:GRP2], idxs_ap=ed2t[:],
                    num_idxs=W2N * P, num_idxs_reg=W2N * P, elem_size=GROW2)
                tile.add_dep_helper(gcall.ins, cc3.ins, sync=True)
                nc.vector.tensor_copy(out=er2S[:],
                                      in_=g2d[:, :, 196:204].bitcast(F32))

                # ================= phase 6: L2 edge phase =================
                def flush2(w, acc):
                    sden = sb.tile([P, 4], F32, tag="sden2")
                    nc.vector.tensor_scalar_max(out=sden[:],
                                                in0=acc[:, 188:192],
                                                scalar1=1e-30)
                    nc.vector.reciprocal(out=sden[:], in_=sden[:])
                    nc.vector.tensor_scalar_mul(out=sden[:], in0=sden[:],
                                                scalar1=0.25)
                    z = sb.tile([P, 188], F32, tag="z2")
                    nc.vector.tensor_tensor(
                        out=z[:].rearrange("p (h c) -> p h c", h=H),
                        in0=acc[:, 0:188].rearrange("p (h c) -> p h c", h=H),
                        in1=sden[:, :, None].broadcast_to([P, H, C]),
                        op=OP.mult)
                    o = sb.tile([P, C], F32, tag="o")
                    nc.vector.tensor_reduce(
                        out=o[:], in_=z[:].rearrange("p (h c) -> p c h", h=H),
                        axis=mybir.AxisListType.X, op=OP.add)
                    if add_b2:
                        nc.vector.tensor_tensor(out=o[:], in0=o[:], in1=b2t[:],
                                                op=OP.add)
                    nc.sync.dma_start(out=OUT[w * P:(w + 1) * P, :], in_=o[:])

                with (
                    tc.tile_pool(name="gp2", bufs=3) as gp2,
                    tc.tile_pool(name="mtp2", bufs=2) as mtp2,
                    tc.tile_pool(name="eep2", bufs=2) as eep2,
                    tc.tile_pool(name="wfp2", bufs=3) as wfp2,
                    tc.tile_pool(name="mp2", bufs=2) as mp2,
                ):
                    edge_phase(l2_calls, t2w, idx2t, mrd2t, MT2,
                               [(G2[0:GRP2], cc3),
                                (G2[GRP2:NCORES * DPC1], cc3)],
                               GROW2, 188, 192, er2S,
                               (gp2, mtp2, eep2, wfp2, mp2), flush2,
                               mtfull=mt2t)

    nc.compile()
    _cache[key] = nc
    return nc


def _run_once(x, W1, al1, ar1, b1, W2, al2, ar2, b2, src0, dst0, src1, dst1):
    def blkdiag(a):  # [H, D] -> [H*D, H]
        out = np.zeros((a.shape[0] * a.shape[1], a.shape[0]), np.float32)
        for h in range(a.shape[0]):
            out[h * a.shape[1]:(h + 1) * a.shape[1], h] = a[h]
        return out

    W1e = np.concatenate([W1, W1 @ blkdiag(al1), W1 @ blkdiag(ar1)],
                         axis=1).astype(BF)
    W2e = np.concatenate([W2, W2 @ blkdiag(al2), W2 @ blkdiag(ar2)],
                         axis=1).astype(BF)
    b1r = np.broadcast_to(b1.reshape(1, 256), (P, 256)).astype(np.float32).copy()
    b2m = b2.reshape(H, C).mean(axis=0)
    b2r = np.broadcast_to(b2m.reshape(1, C), (P, C)).astype(np.float32).copy()
    c2 = W2e.astype(np.float32).sum(axis=0)
    c2r = np.broadcast_to(c2.reshape(1, 196), (P, 196)).astype(np.float32).copy()
    add_b1 = bool(np.any(b1))
    add_b2 = bool(np.any(b2))

    row1 = _g1_row(src0)
    chunk1 = row1 // GRP1
    loc1 = row1 % GRP1
    g2row = _g2_row(src1)
    chunk2 = g2row // GRP2
    loc2 = g2row % GRP2
    core1 = dst0 // BLK1
    core2 = dst1 // BLK2

    in_maps = []
    all_l1_calls = []
    all_l2_calls = []
    packs = []
    for r in range(NCORES):
        sel1 = core1 == r
        c1, i1, m1, t1 = _pack_layer(chunk1[sel1], loc1[sel1],
                                     dst0[sel1] - r * BLK1, W1N, 4)
        sel2 = core2 == r
        c2, i2, m2, t2 = _pack_layer(chunk2[sel2], loc2[sel2],
                                     dst1[sel2] - r * BLK2, W2N, 2)
        all_l1_calls.append(c1)
        all_l2_calls.append(c2)
        packs.append((i1, m1, t1, i2, m2, t2))

    # SPMD: every core runs the same program -> merge call structures by
    # taking, per (window, group), the max tile count across cores.
    def merge_calls(percore, n_win, ngrp):
        merged = []
        for w in range(n_win):
            wc = []
            for gi in range(ngrp):
                nt = 0
                for c in percore:
                    for g_, n_ in c[w]:
                        if g_ == gi:
                            nt = max(nt, n_)
                if nt:
                    wc.append((gi, nt))
            if not wc:
                wc.append((0, 1))
            merged.append(wc)
        return merged

    l1_calls = merge_calls(all_l1_calls, W1N, 4)
    l2_calls = merge_calls(all_l2_calls, W2N, 2)

    # repack per core to the merged structure (pad missing tiles)
    def repack(core_calls, merged, idxc, mrd, mt):
        T = sum(nt for wc in merged for _, nt in wc)
        idx_o = np.zeros((P, 8 * T), np.int16)
        mrd_o = np.full((P, T), 255.0, BF)
        mt_o = np.zeros((P, T * P), BF)
        src_t = 0
        src_map = {}  # (w, g) -> (tile offset, ntiles)
        for w, wc in enumerate(core_calls):
            for g_, n_ in wc:
                src_map[(w, g_)] = (src_t, n_)
                src_t += n_
        dst_t = 0
        for w, wc in enumerate(merged):
            for g_, n_ in wc:
                if (w, g_) in src_map:
                    s0, sn = src_map[(w, g_)]
                    idx_o[:, 8 * dst_t:8 * (dst_t + sn)] = \
                        idxc[:, 8 * s0:8 * (s0 + sn)]
                    mrd_o[:, dst_t:dst_t + sn] = mrd[:, s0:s0 + sn]
                    mt_o[:, P * dst_t:P * (dst_t + sn)] = \
                        mt[:, P * s0:P * (s0 + sn)]
                dst_t += n_
        return idx_o, mrd_o, mt_o

    for r in range(NCORES):
        i1, m1, t1, i2, m2, t2 = packs[r]
        I1, M1, T1m = repack(all_l1_calls[r], l1_calls, i1, m1, t1)
        I2, M2, T2m = repack(all_l2_calls[r], l2_calls, i2, m2, t2)
        # er2 row gather indices: dst2 slot (w,p) -> G2 row of node
        d = np.minimum(1000 * r + np.arange(DPC2), N1 - 1)
        rows = _g2_row(d)
        assert rows.max() < GRP2
        ed = np.zeros((16, 8 * W2N), np.int16)
        for i in range(DPC2):
            ed[i % 16, i // 16] = rows[i]
        ed = np.tile(ed, (8, 1))
        edh = np.zeros((16, DPC1 // 16), np.int16)
        for i in range(DPC1):
            edh[i % 16, i // 16] = i
        edh = np.tile(edh, (8, 1))
        rowsA = np.arange(r * BLK1, (r + 1) * BLK1)
        rowsB = np.arange(N1 + r * BLK1, N1 + (r + 1) * BLK1)
        xT_ = np.zeros((F_IN, LP1), BF)
        xT_[:, :LPC1] = np.concatenate(
            [x[rowsA], x[rowsB]]).T.astype(BF)
        in_maps.append(dict(
            xT=xT_, W1e=W1e, W2e=W2e, b1r=b1r, b2r=b2r, c2r=c2r,
            IDX1=I1, MRD1=M1, MT1=T1m, IDX2=I2, MRD2=M2, MT2=T2m, ED2=ed,
            EDH=edh))

    global _last_in_maps
    _last_in_maps = in_maps
    nc = build_program(l1_calls, l2_calls, add_b1, add_b2)
    from concourse.bass_utils import run_bass_kernel_spmd
    res = None
    last_err = None
    for attempt in range(3):
        try:
            res = run_bass_kernel_spmd(nc, in_maps, core_ids=list(range(NCORES)))
            out = np.concatenate(
                [res.results[r]["OUT"][:BLK2] for r in range(NCORES)], axis=0)
            if np.isnan(out).any() or np.isinf(out).any():
                raise FloatingPointError("nan/inf in kernel output")
            return out.astype(np.float32)
        except Exception as e:
            last_err = e
            import time as _t
            _t.sleep(5)
    raise last_err


def kernel(x, W1, al1, ar1, b1, W2, al2, ar2, b2, src0, dst0, src1, dst1):
    x = np.asarray(x, np.float32); W1 = np.asarray(W1, np.float32)
    al1 = np.asarray(al1, np.float32); ar1 = np.asarray(ar1, np.float32)
    b1 = np.asarray(b1, np.float32); W2 = np.asarray(W2, np.float32)
    al2 = np.asarray(al2, np.float32); ar2 = np.asarray(ar2, np.float32)
    b2 = np.asarray(b2, np.float32)
    src0 = np.asarray(src0, np.int32); dst0 = np.asarray(dst0, np.int32)
    src1 = np.asarray(src1, np.int32); dst1 = np.asarray(dst1, np.int32)
    return _run_once(x, W1, al1, ar1, b1, W2, al2, ar2, b2,
                     src0, dst0, src1, dst1)


# revision 30
# speedup vs baseline: 1.1691x; 1.0177x over previous
"""DGL-style 2-layer GAT on 8 TRN2 NeuronCores (Bass/Tile), v2.

Sharding: dst nodes + incident edges partitioned across 8 cores; weights
replicated; src features shared via AllGather of G (feat rows).

v2 vs baseline: the per-tile indirect DMAs (994ns SWDGE overhead each,
~1.6ms of serialized GPSIMD) are replaced by batched dma_gather
(InstDMAGatherAnt) — one instruction per (window, row-group) gathering a
whole window of 128-edge tiles.  G rows are 768B (256 feat bf16 + 4 el
f32 + pad) to satisfy dma_gather's 256B-multiple row constraint; int16
gather indices force a split of the node table into <=32768-row groups.
The per-edge er gather is gone entirely: er values live in SBUF per dst
window (computed in phase 1 / gathered once for L2) and are aligned to
edge lanes with a per-tile matmul against a host-precomputed transposed
indicator Mt.  Edge-softmax masking is folded into the aggregation
indicator M (pad edges get rd=255 -> zero row).
"""
import sys
sys.path.insert(0, '/opt/trn_rl_repo')

import numpy as np
import ml_dtypes

import concourse.bass as bass
import concourse.tile as tile
from concourse import bacc, mybir, library_config
from concourse.masks import make_identity

P = 128
NCORES = 8
N0, N1, N2 = 100000, 50000, 8000
E0, E1 = 600000, 80000
F_IN, HID, H, C = 256, 64, 4, 47
NEG = 0.2

BLK1 = N1 // NCORES            # 6250  A/B block size
LPC1 = 2 * BLK1                # 12500 nodes owned per core
LP1 = 12544                    # padded to 98*128
W1N = 49                       # L1 windows per core (6272 dst slots)
DPC1 = W1N * P                 # 6272
BLK2 = N2 // NCORES            # 1000 dst2 per core
W2N = 8                        # L2 windows per core (1024 slots)
DPC2 = W2N * P                 # 1024
GROW1 = 384                    # bf16 slots: 256 feat | 8 el-bitcast | 120 pad (768B)
GROW2 = 256                    # 188 feat | 8 el2 | 8 er2 | 52 pad (512B)
GRP1 = 25088                   # G row groups (4 core-pair blocks)
GRP2 = 32768                   # G2 row groups: [0,32768) and [32768,50176)
Q1START = [0, 3200, 6400, 9472]
Q1SIZE = [3200, 3200, 3072, 3072]

F32 = mybir.dt.float32
BF16 = mybir.dt.bfloat16
I16 = mybir.dt.int16
AF = mybir.ActivationFunctionType
OP = mybir.AluOpType
BF = ml_dtypes.bfloat16

_cache = {}
_last_in_maps = None


def _g1_row(n):
    """Global node id (layer1 src space, 0..N0) -> G row."""
    m = n % N1
    r = m // BLK1
    return LP1 * r + (m - BLK1 * r) + np.where(n < N1, 0, BLK1)


def _g2_row(n):
    """node id (layer2 src space, 0..N1) -> G2 row."""
    r = n // BLK1
    return DPC1 * r + (n - BLK1 * r)


def _pack_layer(g, loc, dst_local, n_win, ngrp):
    """Pack one core's edges of one layer into gather calls.

    g/loc: per-edge gather group id and group-local row.
    Returns (calls, idx_cols, mrd, mt):
      calls: per window list of (group, ntiles)
      idx_cols: [128, 8*Ttot] int16 wrapped gather indices
      mrd:  [P, Ttot] lane->dst-lane (255 for pads)
      mt:   [P, Ttot*128] transposed indicator (d x e), pads zero
    """
    w = dst_local // P
    rd = (dst_local % P).astype(np.int64)
    loc = loc.astype(np.int64)
    order = np.lexsort((g, w))
    w, rd, g, loc = w[order], rd[order], g[order], loc[order]
    calls = []
    idx_chunks = []
    rd_chunks = []
    for wi in range(n_win):
        wcalls = []
        sel_w = w == wi
        if not sel_w.any():
            wcalls.append((0, 1))
            idx_chunks.append(np.zeros(P, np.int64))
            rd_chunks.append(np.full(P, 255, np.int64))
            calls.append(wcalls)
            continue
        for gi in range(ngrp):
            sel = sel_w & (g == gi)
            n = int(sel.sum())
            if n == 0:
                continue
            nt = (n + P - 1) // P
            cap = nt * P
            bi = np.zeros(cap, np.int64)
            bi[:n] = loc[sel]
            br = np.full(cap, 255, np.int64)
            br[:n] = rd[sel]
            wcalls.append((gi, nt))
            idx_chunks.append(bi)
            rd_chunks.append(br)
        calls.append(wcalls)
    idx_flat = np.concatenate(idx_chunks)
    rd_flat = np.concatenate(rd_chunks)
    ttot = len(rd_flat) // P
    # idx wrap: per call, flat i -> [i%16, coloff + i//16]; calls are
    # contiguous col ranges, so the global wrap is per-P*nt chunk -- but the
    # wrap granularity is 16, and each call's cols = nt*8.  Since every call
    # length is a multiple of 128 (>=16), wrapping the whole flat array in
    # one pass per call boundary is identical to wrapping chunks.
    idx_cols = np.zeros((16, ttot * 8), np.int16)
    col0 = 0
    pos = 0
    for wcalls in calls:
        for gi, nt in wcalls:
            nidx = nt * P
            chunk = idx_flat[pos:pos + nidx]
            idx_cols[:, col0:col0 + nidx // 16] = chunk.reshape(nidx // 16, 16).T
            pos += nidx
            col0 += nidx // 16
    idx_cols = np.tile(idx_cols, (8, 1))
    mrd = rd_flat.reshape(ttot, P).T.astype(BF)
    # mt[d, t*128+e] = 1 if rd[t,e]==d
    mt = (rd_flat.reshape(ttot, P)[None, :, :]
          == np.arange(P, dtype=np.int64)[:, None, None]).astype(BF)
    mt = mt.reshape(P, ttot * P)
    return calls, idx_cols, mrd, mt


def build_program(l1_calls, l2_calls, add_b1, add_b2):
    key = (tuple(tuple(wc) for wc in l1_calls),
           tuple(tuple(wc) for wc in l2_calls), add_b1, add_b2)
    if key in _cache:
        return _cache[key]
    t1w = [sum(nt for _, nt in wc) for wc in l1_calls]
    t2w = [sum(nt for _, nt in wc) for wc in l2_calls]
    T1 = sum(t1w)
    T2 = sum(t2w)
    maxT = max(max(t1w), max(t2w))
    nc = bacc.Bacc("TRN2", num_devices=NCORES, num_swdge_queues=4)
    # ---- I/O
    xT = nc.declare_dram_parameter("xT", [F_IN, LP1], BF16, isOutput=False)
    W1e = nc.declare_dram_parameter("W1e", [F_IN, 264], BF16, isOutput=False)
    W2e = nc.declare_dram_parameter("W2e", [F_IN, 196], BF16, isOutput=False)
    b1r = nc.declare_dram_parameter("b1r", [P, 256], F32, isOutput=False)
    b2r = nc.declare_dram_parameter("b2r", [P, C], F32, isOutput=False)
    c2r = nc.declare_dram_parameter("c2r", [P, 196], F32, isOutput=False)
    IDX1 = nc.declare_dram_parameter("IDX1", [P, 8 * T1], I16, isOutput=False)
    MRD1 = nc.declare_dram_parameter("MRD1", [P, T1], BF16, isOutput=False)
    MT1 = nc.declare_dram_parameter("MT1", [P, T1 * P], BF16, isOutput=False)
    IDX2 = nc.declare_dram_parameter("IDX2", [P, 8 * T2], I16, isOutput=False)
    MRD2 = nc.declare_dram_parameter("MRD2", [P, T2], BF16, isOutput=False)
    MT2 = nc.declare_dram_parameter("MT2", [P, T2 * P], BF16, isOutput=False)
    ED2 = nc.declare_dram_parameter("ED2", [P, 8 * W2N], I16, isOutput=False)
    EDH = nc.declare_dram_parameter("EDH", [P, DPC1 // 16], I16, isOutput=False)
    OUT = nc.declare_dram_parameter("OUT", [DPC2, C], F32, isOutput=True)
    # ---- internal DRAM
    Gin = nc.dram_tensor("Gin", [LP1, GROW1], BF16)
    G = nc.dram_tensor("G", [NCORES * LP1, GROW1], BF16, addr_space="Shared")
    G2in = nc.dram_tensor("G2in", [DPC1, GROW2], BF16)
    G2 = nc.dram_tensor("G2", [NCORES * DPC1, GROW2], BF16, addr_space="Shared")

    with tile.TileContext(nc) as tc:
        with (
            tc.tile_pool(name="const", bufs=1) as const,
            tc.tile_pool(name="ps", bufs=2, space="PSUM") as ps,
            tc.tile_pool(name="ps2", bufs=2, space="PSUM") as ps2,
            tc.tile_pool(name="sb", bufs=4) as sb,
        ):
            nc.gpsimd.load_library(library_config.mlp)
            iota_i = const.tile([P, maxT, P], mybir.dt.int32)
            nc.gpsimd.iota(iota_i[:], pattern=[[0, maxT], [1, P]], base=0,
                           channel_multiplier=0)
            iotaT = const.tile([P, maxT, P], BF16)
            nc.vector.tensor_copy(out=iotaT[:], in_=iota_i[:])
            ident = const.tile([P, P], BF16)
            make_identity(nc, ident[:])
            w1t = [const.tile([P, 264], BF16, name=f'w1t{k}') for k in range(2)]
            w2t = [const.tile([P, 196], BF16, name=f'w2t{k}') for k in range(2)]
            for k in range(2):
                nc.sync.dma_start(out=w1t[k][:], in_=W1e[k * P:(k + 1) * P, :])
                nc.sync.dma_start(out=w2t[k][:], in_=W2e[k * P:(k + 1) * P, :])
            b1t = const.tile([P, 256], F32)
            nc.sync.dma_start(out=b1t[:], in_=b1r[:])
            b2t = const.tile([P, C], F32)
            nc.sync.dma_start(out=b2t[:], in_=b2r[:])
            c2t = const.tile([P, 196], F32)
            nc.sync.dma_start(out=c2t[:], in_=c2r[:])
            idx1t = const.tile([P, 8 * T1], I16)
            nc.sync.dma_start(out=idx1t[:], in_=IDX1[:])
            mrd1t = const.tile([P, T1], BF16)
            nc.sync.dma_start(out=mrd1t[:], in_=MRD1[:])
            idx2t = const.tile([P, 8 * T2], I16)
            nc.sync.dma_start(out=idx2t[:], in_=IDX2[:])
            mrd2t = const.tile([P, T2], BF16)
            nc.sync.dma_start(out=mrd2t[:], in_=MRD2[:])
            mt2t = const.tile([P, T2 * P], BF16)
            nc.sync.dma_start(out=mt2t[:], in_=MT2[:])
            ed2t = const.tile([P, 8 * W2N], I16)
            nc.sync.dma_start(out=ed2t[:], in_=ED2[:])
            erS = const.tile([P, W1N, 4], BF16)
            er2S = const.tile([P, W2N, 4], BF16)

            # ================= phase 1: feat1 = x @ W1e =================
            g_writes = [[] for _ in range(4)]
            with tc.tile_pool(name="xp", bufs=1) as xp:
                xtq = [[xp.tile([P, Q1SIZE[q]], BF16, name=f'xt{k}q{q}')
                        for q in range(4)] for k in range(2)]
                for k in range(2):
                    for q in range(4):
                        nc.sync.dma_start(
                            out=xtq[k][q][:],
                            in_=xT[k * P:(k + 1) * P,
                                   Q1START[q]:Q1START[q] + Q1SIZE[q]])
                for c in range(LP1 // P):
                    q = 0
                    while c * P >= Q1START[q] + Q1SIZE[q]:
                        q += 1
                    cq = c - Q1START[q] // P
                    pm = ps.tile([P, 264], F32, tag="pfeat")
                    for k in range(2):
                        nc.tensor.matmul(out=pm[:],
                                         lhsT=xtq[k][q][:, cq * P:(cq + 1) * P],
                                         rhs=w1t[k][:],
                                         start=(k == 0), stop=(k == 1))
                    gs = sb.tile([P, GROW1], BF16, tag="gs")
                    nc.vector.tensor_copy(out=gs[:, 0:256], in_=pm[:, 0:256])
                    nc.vector.tensor_copy(
                        out=gs[:, 256:264].bitcast(F32), in_=pm[:, 256:260])
                    d1 = nc.sync.dma_start(out=Gin[c * P:(c + 1) * P, :],
                                           in_=gs[:, :])
                    g_writes[q].append(d1)
                    if c < W1N:
                        nc.vector.tensor_copy(out=erS[:, c, :], in_=pm[:, 260:264])

            # ================= phase 2: AllGather G =================
            cc1 = nc.gpsimd.collective_compute(
                "AllGather", OP.bypass, replica_groups=[list(range(NCORES))],
                ins=[Gin[:]], outs=[G[:]])
            for q in range(4):
                for d in g_writes[q]:
                    tile.add_dep_helper(cc1.ins, d.ins, sync=True)

            # ============ shared edge-phase body ============
            def hoist_er(calls, tws, MTp, ers, mtp, erEall):
                """er-edge alignment for all windows; independent of the
                AllGather, so the PE does it during the collective wait."""
                wt0 = 0
                for w, wcalls in enumerate(calls):
                    T = tws[w]
                    mts = mtp.tile([P, T * P], BF16, tag="mt")
                    nc.sync.dma_start(out=mts[:],
                                      in_=MTp[:, wt0 * P:(wt0 + T) * P])
                    erPS = ps2.tile([P, T, 4], F32, tag="erps")
                    for j in range(T):
                        nc.tensor.matmul(out=erPS[:, j, :],
                                         lhsT=mts[:, j * P:(j + 1) * P],
                                         rhs=ers[:, w, :],
                                         start=True, stop=True)
                    nc.scalar.activation(out=erEall[:, wt0:wt0 + T, :],
                                          in_=erPS[:], func=AF.Copy)
                    wt0 += T

            def edge_phase(calls, tws, idxt, mrdt, MTp, gtabs, grow,
                           nfeat, acc_cols, ers, pools, flush_fn, erEall=None,
                           mtfull=None):
                gp, mtp, eep, wfp, mp = pools
                wt0 = 0   # running tile offset
                qn = 0
                for w, wcalls in enumerate(calls):
                    T = tws[w]
                    if mtfull is not None:
                        mts = mtfull[:, wt0 * P:(wt0 + T) * P]
                    elif erEall is None:
                        mts = mtp.tile([P, T * P], BF16, tag="mt")
                        nc.sync.dma_start(out=mts[:],
                                          in_=MTp[:, wt0 * P:(wt0 + T) * P])
                    gb = gp.tile([P, T, grow], BF16, tag="gb")
                    t0 = 0
                    for gi, nt in wcalls:
                        gtab, gdep = gtabs[gi]
                        gcall = nc.gpsimd.dma_gather(
                            out_ap=gb[:, t0:t0 + nt, :],
                            in_ap=gtab,
                            idxs_ap=idxt[:, 8 * (wt0 + t0):8 * (wt0 + t0 + nt)],
                            num_idxs=nt * P, num_idxs_reg=nt * P,
                            elem_size=grow, queue_num=qn % 4)
                        qn += 1
                        tile.add_dep_helper(gcall.ins, gdep.ins, sync=True)
                        t0 += nt
                    if erEall is None:
                        # er alignment in-loop: erE[:, j, :] = Mt_j @ er_win
                        erE = ps2.tile([P, T, 4], F32, tag="erps")
                        for j in range(T):
                            nc.tensor.matmul(out=erE[:, j, :],
                                             lhsT=mts[:, j * P:(j + 1) * P],
                                             rhs=ers[:, w, :],
                                             start=True, stop=True)
                        erE = erE[:]
                    else:
                        erE = erEall[:, wt0:wt0 + T, :]
                    eef = eep.tile([P, T, 4], F32, tag="eef")
                    nc.vector.tensor_tensor(
                        out=eef[:],
                        in0=gb[:, :, nfeat:nfeat + 8].bitcast(F32),
                        in1=erE, op=OP.add)
                    # exp(lrelu(x)) == max(exp(x), exp(0.2x)) exactly
                    ex1 = eep.tile([P, T, 4], F32, tag="ex1")
                    nc.scalar.activation(out=ex1[:], in_=eef[:], func=AF.Exp)
                    ex2 = eep.tile([P, T, 4], F32, tag="ex2")
                    nc.scalar.activation(out=ex2[:], in_=eef[:], func=AF.Exp,
                                         scale=NEG)
                    ees = wfp.tile([P, T, nfeat + 4], BF16, tag="ees")
                    nc.vector.tensor_tensor(out=ees[:, :, nfeat:nfeat + 4],
                                            in0=ex1[:], in1=ex2[:], op=OP.max)
                    hd = nfeat // H
                    for h in range(H):
                        nc.vector.tensor_tensor(
                            out=ees[:, :, h * hd:(h + 1) * hd],
                            in0=gb[:, :, h * hd:(h + 1) * hd],
                            in1=ees[:, :, nfeat + h:nfeat + h + 1].broadcast_to(
                                [P, T, hd]),
                            op=OP.mult)
                    mall = mp.tile([P, T, P], BF16, tag="mall")
                    nc.vector.tensor_tensor(
                        out=mall[:], in0=iotaT[:, 0:T, :],
                        in1=mrdt[:, wt0:wt0 + T][:, :, None].broadcast_to(
                            [P, T, P]),
                        op=OP.is_equal)
                    acc = ps.tile([P, acc_cols], F32, tag="acc")
                    for j in range(T):
                        nc.tensor.matmul(out=acc[:], lhsT=mall[:, j, :],
                                         rhs=ees[:, j, :],
                                         start=(j == 0), stop=(j == T - 1))
                    flush_fn(w, acc)
                    wt0 += T

            # ================= phase 3: L1 edge phase =================
            hT = [const.tile([P, DPC1], BF16, name=f'hT{k}') for k in range(2)]
            g2_writes = []

            def flush1(w, acc):
                sden = sb.tile([P, 4], F32, tag="sden")
                nc.vector.tensor_scalar_max(out=sden[:], in0=acc[:, 256:260],
                                            scalar1=1e-30)
                nc.vector.reciprocal(out=sden[:], in_=sden[:])
                z = sb.tile([P, 256], BF16, tag="z")
                nc.vector.tensor_tensor(
                    out=z[:].rearrange("p (h d) -> p h d", h=H),
                    in0=acc[:, 0:256].rearrange("p (h d) -> p h d", h=H),
                    in1=sden[:, :, None].broadcast_to([P, H, HID]), op=OP.mult)
                if add_b1:
                    nc.vector.tensor_tensor(out=z[:], in0=z[:], in1=b1t[:],
                                            op=OP.add)
                # store h+1 = elu(z)+1 = relu(z) + exp(-relu(-z)); the -1
                # is folded into phase 4 as a W2e column-sum correction.
                zm = sb.tile([P, 256], BF16, tag="zm")
                nc.scalar.activation(out=zm[:], in_=z[:], func=AF.Relu,
                                     scale=-1.0)
                nc.scalar.activation(out=zm[:], in_=zm[:], func=AF.Exp,
                                     scale=-1.0)
                hb = sb.tile([P, 256], BF16, tag="hb")
                nc.scalar.activation(out=hb[:], in_=z[:], func=AF.Relu)
                nc.vector.tensor_tensor(out=hb[:], in0=hb[:], in1=zm[:],
                                        op=OP.add)
                for k in range(2):
                    tp = ps.tile([P, P], BF16, tag="tp")
                    nc.tensor.transpose(out=tp[:], in_=hb[:, k * P:(k + 1) * P],
                                        identity=ident[:])
                    nc.vector.tensor_copy(out=hT[k][:, w * P:(w + 1) * P],
                                          in_=tp[:])
                # fused phase 4: feat2 row block for this window
                pm2 = ps.tile([P, 264], F32, tag="pfeat")
                for k in range(2):
                    nc.tensor.matmul(out=pm2[:, 0:196],
                                     lhsT=hT[k][:, w * P:(w + 1) * P],
                                     rhs=w2t[k][:],
                                     start=(k == 0), stop=(k == 1))
                gs2 = sb.tile([P, GROW2], BF16, tag="gs2")
                nc.vector.tensor_tensor(out=gs2[:, 0:188], in0=pm2[:, 0:188],
                                        in1=c2t[:, 0:188], op=OP.subtract)
                nc.vector.tensor_tensor(
                    out=gs2[:, 188:204].bitcast(F32), in0=pm2[:, 188:196],
                    in1=c2t[:, 188:196], op=OP.subtract)
                d1 = nc.sync.dma_start(out=G2in[w * P:(w + 1) * P, :],
                                       in_=gs2[:, :])
                g2_writes.append(d1)

            with (
                tc.tile_pool(name="gp", bufs=4) as gp,
                tc.tile_pool(name="mtp", bufs=2) as mtp,
                tc.tile_pool(name="eep", bufs=3) as eep,
                tc.tile_pool(name="wfp", bufs=4) as wfp,
                tc.tile_pool(name="mp", bufs=3) as mp,
            ):
                gsz1 = [GRP1, GRP1, GRP1, NCORES * LP1 - 3 * GRP1]
                edge_phase(l1_calls, t1w, idx1t, mrd1t, MT1,
                           [(G[q * GRP1:q * GRP1 + gsz1[q]], cc1)
                            for q in range(4)],
                           GROW1, 256, 260, erS,
                           (gp, mtp, eep, wfp, mp), flush1)

            # ================= phase 5: AllGather G2 =================
            cc3 = nc.gpsimd.collective_compute(
                "AllGather", OP.bypass, replica_groups=[list(range(NCORES))],
                ins=[G2in[:]], outs=[G2[:]])
            for d in g2_writes:
                tile.add_dep_helper(cc3.ins, d.ins, sync=True)

            # ===== phase 5b: er2 for my dst2 rows (one gather from G2) =====
            with tc.tile_pool(name="e2p", bufs=1) as e2p:
                g2d = e2p.tile([P, W2N, GROW2], BF16)
                gcall = nc.gpsimd.dma_gather(
                    out_ap=g2d[:], in_ap=G2[0:GRP2], idxs_ap=ed2t[:],
                    num_idxs=W2N * P, num_idxs_reg=W2N * P, elem_size=GROW2)
                tile.add_dep_helper(gcall.ins, cc3.ins, sync=True)
                nc.vector.tensor_copy(out=er2S[:],
                                      in_=g2d[:, :, 196:204].bitcast(F32))

                # ================= phase 6: L2 edge phase =================
                def flush2(w, acc):
                    sden = sb.tile([P, 4], F32, tag="sden2")
                    nc.vector.tensor_scalar_max(out=sden[:],
                                                in0=acc[:, 188:192],
                                                scalar1=1e-30)
                    nc.vector.reciprocal(out=sden[:], in_=sden[:])
                    nc.vector.tensor_scalar_mul(out=sden[:], in0=sden[:],
                                                scalar1=0.25)
                    z = sb.tile([P, 188], F32, tag="z2")
                    nc.vector.tensor_tensor(
                        out=z[:].rearrange("p (h c) -> p h c", h=H),
                        in0=acc[:, 0:188].rearrange("p (h c) -> p h c", h=H),
                        in1=sden[:, :, None].broadcast_to([P, H, C]),
                        op=OP.mult)
                    o = sb.tile([P, C], F32, tag="o")
                    nc.vector.tensor_reduce(
                        out=o[:], in_=z[:].rearrange("p (h c) -> p c h", h=H),
                        axis=mybir.AxisListType.X, op=OP.add)
                    if add_b2:
                        nc.vector.tensor_tensor(out=o[:], in0=o[:], in1=b2t[:],
                                                op=OP.add)
                    nc.sync.dma_start(out=OUT[w * P:(w + 1) * P, :], in_=o[:])

                with (
                    tc.tile_pool(name="gp2", bufs=3) as gp2,
                    tc.tile_pool(name="mtp2", bufs=2) as mtp2,
                    tc.tile_pool(name="eep2", bufs=2) as eep2,
                    tc.tile_pool(name="wfp2", bufs=3) as wfp2,
                    tc.tile_pool(name="mp2", bufs=2) as mp2,
                ):
                    edge_phase(l2_calls, t2w, idx2t, mrd2t, MT2,
                               [(G2[0:GRP2], cc3),
                                (G2[GRP2:NCORES * DPC1], cc3)],
                               GROW2, 188, 192, er2S,
                               (gp2, mtp2, eep2, wfp2, mp2), flush2,
                               mtfull=mt2t)

    nc.compile()
    _cache[key] = nc
    return nc


def _run_once(x, W1, al1, ar1, b1, W2, al2, ar2, b2, src0, dst0, src1, dst1):
    def blkdiag(a):  # [H, D] -> [H*D, H]
        out = np.zeros((a.shape[0] * a.shape[1], a.shape[0]), np.float32)
        for h in range(a.shape[0]):
            out[h * a.shape[1]:(h + 1) * a.shape[1], h] = a[h]
        return out

    W1e = np.concatenate([W1, W1 @ blkdiag(al1), W1 @ blkdiag(ar1)],
                         axis=1).astype(BF)
    W2e = np.concatenate([W2, W2 @ blkdiag(al2), W2 @ blkdiag(ar2)],
                         axis=1).astype(BF)
    b1r = np.broadcast_to(b1.reshape(1, 256), (P, 256)).astype(np.float32).copy()
    b2m = b2.reshape(H, C).mean(axis=0)
    b2r = np.broadcast_to(b2m.reshape(1, C), (P, C)).astype(np.float32).copy()
    c2 = W2e.astype(np.float32).sum(axis=0)
    c2r = np.broadcast_to(c2.reshape(1, 196), (P, 196)).astype(np.float32).copy()
    add_b1 = bool(np.any(b1))
    add_b2 = bool(np.any(b2))

    row1 = _g1_row(src0)
    chunk1 = row1 // GRP1
    loc1 = row1 % GRP1
    g2row = _g2_row(src1)
    chunk2 = g2row // GRP2
    loc2 = g2row % GRP2
    core1 = dst0 // BLK1
    core2 = dst1 // BLK2

    in_maps = []
    all_l1_calls = []
    all_l2_calls = []
    packs = []
    for r in range(NCORES):
        sel1 = core1 == r
        c1, i1, m1, t1 = _pack_layer(chunk1[sel1], loc1[sel1],
                                     dst0[sel1] - r * BLK1, W1N, 4)
        sel2 = core2 == r
        c2, i2, m2, t2 = _pack_layer(chunk2[sel2], loc2[sel2],
                                     dst1[sel2] - r * BLK2, W2N, 2)
        all_l1_calls.append(c1)
        all_l2_calls.append(c2)
        packs.append((i1, m1, t1, i2, m2, t2))

    # SPMD: every core runs the same program -> merge call structures by
    # taking, per (window, group), the max tile count across cores.
    def merge_calls(percore, n_win, ngrp):
        merged = []
        for w in range(n_win):
            wc = []
            for gi in range(ngrp):
                nt = 0
                for c in percore:
                    for g_, n_ in c[w]:
                        if g_ == gi:
                            nt = max(nt, n_)
                if nt:
                    wc.append((gi, nt))
            if not wc:
                wc.append((0, 1))
            merged.append(wc)
        return merged

    l1_calls = merge_calls(all_l1_calls, W1N, 4)
    l2_calls = merge_calls(all_l2_calls, W2N, 2)

    # repack per core to the merged structure (pad missing tiles)
    def repack(core_calls, merged, idxc, mrd, mt):
        T = sum(nt for wc in merged for _, nt in wc)
        idx_o = np.zeros((P, 8 * T), np.int16)
        mrd_o = np.full((P, T), 255.0, BF)
        mt_o = np.zeros((P, T * P), BF)
        src_t = 0
        src_map = {}  # (w, g) -> (tile offset, ntiles)
        for w, wc in enumerate(core_calls):
            for g_, n_ in wc:
                src_map[(w, g_)] = (src_t, n_)
                src_t += n_
        dst_t = 0
        for w, wc in enumerate(merged):
            for g_, n_ in wc:
                if (w, g_) in src_map:
                    s0, sn = src_map[(w, g_)]
                    idx_o[:, 8 * dst_t:8 * (dst_t + sn)] = \
                        idxc[:, 8 * s0:8 * (s0 + sn)]
                    mrd_o[:, dst_t:dst_t + sn] = mrd[:, s0:s0 + sn]
                    mt_o[:, P * dst_t:P * (dst_t + sn)] = \
                        mt[:, P * s0:P * (s0 + sn)]
                dst_t += n_
        return idx_o, mrd_o, mt_o

    for r in range(NCORES):
        i1, m1, t1, i2, m2, t2 = packs[r]
        I1, M1, T1m = repack(all_l1_calls[r], l1_calls, i1, m1, t1)
        I2, M2, T2m = repack(all_l2_calls[r], l2_calls, i2, m2, t2)
        # er2 row gather indices: dst2 slot (w,p) -> G2 row of node
        d = np.minimum(1000 * r + np.arange(DPC2), N1 - 1)
        rows = _g2_row(d)
        assert rows.max() < GRP2
        ed = np.zeros((16, 8 * W2N), np.int16)
        for i in range(DPC2):
            ed[i % 16, i // 16] = rows[i]
        ed = np.tile(ed, (8, 1))
        edh = np.zeros((16, DPC1 // 16), np.int16)
        for i in range(DPC1):
            edh[i % 16, i // 16] = i
        edh = np.tile(edh, (8, 1))
        rowsA = np.arange(r * BLK1, (r + 1) * BLK1)
        rowsB = np.arange(N1 + r * BLK1, N1 + (r + 1) * BLK1)
        xT_ = np.zeros((F_IN, LP1), BF)
        xT_[:, :LPC1] = np.concatenate(
            [x[rowsA], x[rowsB]]).T.astype(BF)
        in_maps.append(dict(
            xT=xT_, W1e=W1e, W2e=W2e, b1r=b1r, b2r=b2r, c2r=c2r,
            IDX1=I1, MRD1=M1, MT1=T1m, IDX2=I2, MRD2=M2, MT2=T2m, ED2=ed,
            EDH=edh))

    global _last_in_maps
    _last_in_maps = in_maps
    nc = build_program(l1_calls, l2_calls, add_b1, add_b2)
    from concourse.bass_utils import run_bass_kernel_spmd
    res = None
    last_err = None
    for attempt in range(3):
        try:
            res = run_bass_kernel_spmd(nc, in_maps, core_ids=list(range(NCORES)))
            out = np.concatenate(
                [res.results[r]["OUT"][:BLK2] for r in range(NCORES)], axis=0)
            if np.isnan(out).any() or np.isinf(out).any():
                raise FloatingPointError("nan/inf in kernel output")
            return out.astype(np.float32)
        except Exception as e:
            last_err = e
            import time as _t
            _t.sleep(5)
    raise last_err


def kernel(x, W1, al1, ar1, b1, W2, al2, ar2, b2, src0, dst0, src1, dst1):
    x = np.asarray(x, np.float32); W1 = np.asarray(W1, np.float32)
    al1 = np.asarray(al1, np.float32); ar1 = np.asarray(ar1, np.float32)
    b1 = np.asarray(b1, np.float32); W2 = np.asarray(W2, np.float32)
    al2 = np.asarray(al2, np.float32); ar2 = np.asarray(ar2, np.float32)
    b2 = np.asarray(b2, np.float32)
    src0 = np.asarray(src0, np.int32); dst0 = np.asarray(dst0, np.int32)
    src1 = np.asarray(src1, np.int32); dst1 = np.asarray(dst1, np.int32)
    return _run_once(x, W1, al1, ar1, b1, W2, al2, ar2, b2,
                     src0, dst0, src1, dst1)
